# revision 7
# baseline (speedup 1.0000x reference)
"""MoCo forward kernel for 8 Trainium2 NeuronCores.

Sharding:
  - Projections (h->q, h->k), l_pos, cls head: data-parallel over batch
    (128 rows per core), followed by an AllGather of raw q vectors.
  - Per-info-class centroid sums: sharded over the queue dim K
    (8192 rows per core) via a one-hot matmul, combined with an AllReduce.
  - The big B x K similarity / mask / MAE phase: sharded over K columns
    (each core computes all 1024 rows against its 8192 queue columns),
    which avoids replicating the 2x 32MB queue reads on every core.

All matmuls run in fp16 (same PE rate as bf16, 4x the mantissa) with fp32
PSUM accumulation; l2 normalization factors are folded into pre-scaled
copies of q so the inner loop is pure matmul + one ACT sqrt + one DVE
copy + one DVE masked-reduce + one GPSIMD compare per tile.
"""

import sys

sys.path.insert(0, '/opt/trn_rl_repo')

from contextlib import ExitStack

import numpy as np

import concourse.bass as bass
import concourse.mybir as mybir
import concourse.tile as tile
from concourse import bacc
from concourse.bass_utils import run_bass_kernel_spmd

NCORES = 8
B, E, D, K = 1024, 2048, 128, 65536
NI, NCLS = 1024, 100
T = 0.07
BS = B // NCORES          # 128 batch rows per core
KS = K // NCORES          # 8192 queue rows/cols per core
CH = 512                  # chunk width in the K loop
NCH = KS // CH            # 16 chunks per core
NBT = B // 128            # 8 batch tiles of 128
ET = E // 128             # 16 contraction tiles over EMBED

F16 = mybir.dt.float16
F32 = mybir.dt.float32
I32 = mybir.dt.int32
AX = mybir.AxisListType
OP = mybir.AluOpType
AF = mybir.ActivationFunctionType


def build_kernel():
    nc = bacc.Bacc('TRN2', target_bir_lowering=False, debug=False,
                   num_devices=NCORES)

    # ---- DRAM I/O ----
    hqT = nc.dram_tensor("hqT", [E, BS], F16, kind="ExternalInput")
    hkT = nc.dram_tensor("hkT", [E, BS], F16, kind="ExternalInput")
    w1 = nc.dram_tensor("w1", [E, E], F16, kind="ExternalInput")
    wk1 = nc.dram_tensor("wk1", [E, E], F16, kind="ExternalInput")
    w2 = nc.dram_tensor("w2", [E, D], F16, kind="ExternalInput")
    wk2 = nc.dram_tensor("wk2", [E, D], F16, kind="ExternalInput")
    wc = nc.dram_tensor("wc", [E, NCLS], F16, kind="ExternalInput")
    bp1 = nc.dram_tensor("bp1", [E], F32, kind="ExternalInput")
    bkp1 = nc.dram_tensor("bkp1", [E], F32, kind="ExternalInput")
    bp2 = nc.dram_tensor("bp2", [D], F32, kind="ExternalInput")
    bkp2 = nc.dram_tensor("bkp2", [D], F32, kind="ExternalInput")
    bcls = nc.dram_tensor("bcls", [NCLS], F32, kind="ExternalInput")
    qslT = nc.dram_tensor("qslT", [D, KS], F16, kind="ExternalInput")
    cslT = nc.dram_tensor("cslT", [D, KS], F16, kind="ExternalInput")
    cnat = nc.dram_tensor("cnat", [KS, D], F16, kind="ExternalInput")
    labs = nc.dram_tensor("labs", [KS], F32, kind="ExternalInput")
    clab = nc.dram_tensor("clab", [BS], F32, kind="ExternalInput")

    lneg = nc.dram_tensor("lneg", [B, KS], F32, kind="ExternalOutput")
    stats = nc.dram_tensor("stats", [BS, 32], F32, kind="ExternalOutput")

    groups = [list(range(NCORES))]

    with tile.TileContext(nc) as tc, ExitStack() as ctx:
        consts = ctx.enter_context(tc.tile_pool(name="consts", bufs=1))
        persist = ctx.enter_context(tc.tile_pool(name="persist", bufs=1))
        dram = ctx.enter_context(tc.tile_pool(name="dram", bufs=1, space="DRAM"))

        # constants
        iota_i = consts.tile([128, NI], F32)
        nc.gpsimd.iota(iota_i, pattern=[[1, NI]], base=0, channel_multiplier=0,
                       allow_small_or_imprecise_dtypes=True)
        ones_col = consts.tile([128, 1], F32)
        nc.vector.memset(ones_col, 1.0)
        ones_row = consts.tile([1, 128], F32)
        nc.vector.memset(ones_row, 1.0)

        stats_sb = persist.tile([128, 32], F32)
        nc.vector.memset(stats_sb, 0.0)
        eps30 = consts.tile([1, 1], F32)
        nc.vector.memset(eps30, 1e-30)
        maebias = consts.tile([128, 1], F32)
        nc.vector.memset(maebias, 2.0 + 1e-6)

        # =========== Phase A: centroid partial sums over own K slice =========
        # cent_ps[d, i] += sum_k copy[k, d] * onehot[k, i]
        cent_sum = persist.tile([128, NI], F32)
        with tc.tile_pool(name="psA", bufs=1, space="PSUM") as psA, \
             tc.tile_pool(name="poolA", bufs=3) as poolA:
            cent_ps0 = psA.tile([128, 512], F32, tag="c0")
            cent_ps1 = psA.tile([128, 512], F32, tag="c1")
            nkt = KS // 128  # 64
            for kt in range(nkt):
                cpt = poolA.tile([128, D], F16, tag="cpt")
                nc.sync.dma_start(out=cpt, in_=cnat[kt * 128:(kt + 1) * 128, :])
                labc = poolA.tile([128, 1], F32, tag="labc")
                nc.sync.dma_start(
                    out=labc,
                    in_=labs[kt * 128:(kt + 1) * 128].rearrange("(p o) -> p o", o=1))
                oh = poolA.tile([128, NI], F16, tag="oh")
                nc.vector.tensor_scalar(oh, iota_i, labc, None, OP.is_equal)
                nc.tensor.matmul(cent_ps0, lhsT=cpt, rhs=oh[:, 0:512],
                                 start=(kt == 0), stop=(kt == nkt - 1))
                nc.tensor.matmul(cent_ps1, lhsT=cpt, rhs=oh[:, 512:NI],
                                 start=(kt == 0), stop=(kt == nkt - 1))
            nc.scalar.copy(cent_sum[:, 0:512], cent_ps0)
            nc.scalar.copy(cent_sum[:, 512:NI], cent_ps1)

        cent_in = dram.tile([128, NI], F32)
        cent_out = dram.tile([128, NI], F32)
        nc.sync.dma_start(out=cent_in, in_=cent_sum)
        nc.gpsimd.collective_compute(
            "AllReduce", OP.add, replica_groups=groups,
            ins=[cent_in.opt()], outs=[cent_out.opt()])
        centall = persist.tile([128, NI], F32)
        nc.sync.dma_start(out=centall, in_=cent_out)

        # =========== Phase B: projections (own 128 batch rows) ==============
        hq_sb = persist.tile([128, ET, BS], F16)
        nc.sync.dma_start(out=hq_sb, in_=hqT.rearrange("(t p) b -> p t b", p=128))
        hk_sb = persist.tile([128, ET, BS], F16)
        nc.sync.dma_start(out=hk_sb, in_=hkT.rearrange("(t p) b -> p t b", p=128))
        w2_sb = persist.tile([128, ET, D], F16)
        nc.sync.dma_start(out=w2_sb, in_=w2.rearrange("(t p) d -> p t d", p=128))
        wk2_sb = persist.tile([128, ET, D], F16)
        nc.sync.dma_start(out=wk2_sb, in_=wk2.rearrange("(t p) d -> p t d", p=128))
        bp1_sb = persist.tile([128, ET], F32)
        nc.sync.dma_start(out=bp1_sb, in_=bp1.rearrange("(t p) -> p t", p=128))
        bkp1_sb = persist.tile([128, ET], F32)
        nc.sync.dma_start(out=bkp1_sb, in_=bkp1.rearrange("(t p) -> p t", p=128))
        bp2_sb = persist.tile([128, 1], F32)
        nc.sync.dma_start(out=bp2_sb, in_=bp2.rearrange("(p o) -> p o", o=1))
        bkp2_sb = persist.tile([128, 1], F32)
        nc.sync.dma_start(out=bkp2_sb, in_=bkp2.rearrange("(p o) -> p o", o=1))

        qraw = persist.tile([128, BS], F32)   # [d, b] fp32, own shard
        kraw = persist.tile([128, BS], F32)

        with tc.tile_pool(name="psB", bufs=1, space="PSUM") as psB, \
             tc.tile_pool(name="poolB", bufs=3) as poolB, \
             tc.tile_pool(name="z1", bufs=1) as z1pool:
            ps_small = psB.tile([128, 16], F32, tag="small")
            for branch in range(2):
                wsrc = w1 if branch == 0 else wk1
                hsb = hq_sb if branch == 0 else hk_sb
                b1sb = bp1_sb if branch == 0 else bkp1_sb
                w2sb = w2_sb if branch == 0 else wk2_sb
                b2sb = bp2_sb if branch == 0 else bkp2_sb
                zout = qraw if branch == 0 else kraw
                z1_sb = z1pool.tile([128, ET, BS], F16, tag=f"z1_{branch}")
                for ot in range(ET):
                    wcol = poolB.tile([128, ET, 128], F16, tag="wcol")
                    nc.sync.dma_start(
                        out=wcol,
                        in_=wsrc[:, ot * 128:(ot + 1) * 128]
                        .rearrange("(t p) m -> p t m", p=128))
                    zps = psB.tile([128, BS], F32, tag="zps")
                    for et in range(ET):
                        nc.tensor.matmul(zps, lhsT=wcol[:, et, :],
                                         rhs=hsb[:, et, :],
                                         start=(et == 0), stop=(et == ET - 1))
                    nc.scalar.activation(z1_sb[:, ot, :], zps, AF.Relu,
                                         bias=b1sb[:, ot:ot + 1], scale=1.0)
                qps = psB.tile([128, BS], F32, tag="zps")
                for et in range(ET):
                    nc.tensor.matmul(qps, lhsT=w2sb[:, et, :], rhs=z1_sb[:, et, :],
                                     start=(et == 0), stop=(et == ET - 1))
                nc.scalar.activation(zout, qps, AF.Identity,
                                     bias=b2sb[:, 0:1], scale=1.0)

            # own-shard norms and l_pos
            sq = poolB.tile([128, BS], F32, tag="sqloc")
            nc.vector.tensor_mul(sq, qraw, qraw)
            nc.tensor.matmul(ps_small[:, 8:9], lhsT=sq, rhs=ones_col)
            sk = poolB.tile([128, BS], F32, tag="sqloc")
            nc.vector.tensor_mul(sk, kraw, kraw)
            nc.tensor.matmul(ps_small[:, 9:10], lhsT=sk, rhs=ones_col)
            pq = poolB.tile([128, BS], F32, tag="sqloc")
            nc.vector.tensor_mul(pq, qraw, kraw)
            nc.tensor.matmul(ps_small[:, 10:11], lhsT=pq, rhs=ones_col)
            nrm2 = persist.tile([128, 4], F32)
            nc.scalar.activation(nrm2[:, 0:2], ps_small[:, 8:10], AF.Sqrt,
                                 bias=0.0, scale=1.0)
            rloc = persist.tile([128, 2], F32)
            nc.vector.reciprocal(rloc, nrm2[:, 0:2])
            # l_pos = (q.k) * rnq * rnk / T  -> stats col 25
            lposv = persist.tile([128, 1], F32)
            nc.vector.tensor_scalar(lposv, ps_small[:, 10:11],
                                    rloc[:, 0:1], rloc[:, 1:2], OP.mult, OP.mult)
            nc.vector.tensor_scalar_mul(stats_sb[:, 25:26], lposv, 1.0 / T)

        # AllGather raw q across cores -> [d, B] on every core
        ag_in = dram.tile([128, BS], F32)
        ag_out = dram.tile([NCORES, 128, BS], F32)
        nc.sync.dma_start(out=ag_in, in_=qraw)
        nc.gpsimd.collective_compute(
            "AllGather", OP.bypass, replica_groups=groups,
            ins=[ag_in.opt()], outs=[ag_out.opt()])
        qall = persist.tile([128, B], F32)
        for c in range(NCORES):
            nc.sync.dma_start(out=qall[:, c * BS:(c + 1) * BS], in_=ag_out[c])

        # global norms (row layout) and pre-scaled fp16 copies of q
        q16q = persist.tile([128, B], F16)   # q * rnq / T      (logits)
        q16c = persist.tile([128, B], F16)   # q * (-2 rnq)     (mae)
        with tc.tile_pool(name="psC", bufs=2, space="PSUM") as psC, \
             tc.tile_pool(name="poolC", bufs=2) as poolC:
            sqall = poolC.tile([128, B], F32, tag="sqall")
            nc.vector.tensor_mul(sqall, qall, qall)
            nrow_ps = psC.tile([1, B], F32, tag="nrow")
            nc.tensor.matmul(nrow_ps[:, 0:512], lhsT=ones_col, rhs=sqall[:, 0:512])
            nc.tensor.matmul(nrow_ps[:, 512:B], lhsT=ones_col, rhs=sqall[:, 512:B])
            nrow = poolC.tile([1, B], F32, tag="nrowsb")
            nc.scalar.activation(nrow[:, 0:512], nrow_ps[:, 0:512], AF.Sqrt,
                                 bias=0.0, scale=1.0)
            nc.scalar.activation(nrow[:, 512:B], nrow_ps[:, 512:B], AF.Sqrt,
                                 bias=0.0, scale=1.0)
            rrow = poolC.tile([1, B], F32, tag="rrow")
            nc.vector.reciprocal(rrow, nrow)
            rq_row = poolC.tile([1, B], F32, tag="rqrow")
            nc.vector.tensor_scalar_mul(rq_row, rrow, 1.0 / T)
            rc_row = poolC.tile([1, B], F32, tag="rcrow")
            nc.vector.tensor_scalar_mul(rc_row, rrow, -2.0)
            for half in range(2):
                sl = slice(half * 512, (half + 1) * 512)
                bq_ps = psC.tile([128, 512], F32, tag="bq")
                nc.tensor.matmul(bq_ps, lhsT=ones_row, rhs=rq_row[:, sl])
                nc.vector.tensor_mul(q16q[:, sl], qall[:, sl], bq_ps)
                bc_ps = psC.tile([128, 512], F32, tag="bq")
                nc.tensor.matmul(bc_ps, lhsT=ones_row, rhs=rc_row[:, sl])
                nc.vector.tensor_mul(q16c[:, sl], qall[:, sl], bc_ps)

            # ========= Phase C: centroid normalize + pseudo labels ==========
            sqc = poolC.tile([128, NI], F32, tag="sqall")
            nc.vector.tensor_mul(sqc, centall, centall)
            cn_ps = psC.tile([1, NI], F32, tag="nrow")
            nc.tensor.matmul(cn_ps[:, 0:512], lhsT=ones_col, rhs=sqc[:, 0:512])
            nc.tensor.matmul(cn_ps[:, 512:NI], lhsT=ones_col, rhs=sqc[:, 512:NI])
            cnn = poolC.tile([1, NI], F32, tag="nrowsb")
            nc.scalar.activation(cnn[:, 0:512], cn_ps[:, 0:512], AF.Sqrt,
                                 bias=eps30[0:1, 0:1], scale=1.0)
            nc.scalar.activation(cnn[:, 512:NI], cn_ps[:, 512:NI], AF.Sqrt,
                                 bias=eps30[0:1, 0:1], scale=1.0)
            crn = poolC.tile([1, NI], F32, tag="rrow")
            nc.vector.reciprocal(crn, cnn)
            centn = persist.tile([128, NI], F16)
            for half in range(2):
                sl = slice(half * 512, (half + 1) * 512)
                cb_ps = psC.tile([128, 512], F32, tag="bq")
                nc.tensor.matmul(cb_ps, lhsT=ones_row, rhs=crn[:, sl])
                nc.vector.tensor_mul(centn[:, sl], centall[:, sl], cb_ps)

            pseudo_i = persist.tile([128, NBT], F32)
            for bt in range(NBT):
                plog = poolC.tile([128, NI], F32, tag="plog")
                for half in range(2):
                    sl = slice(half * 512, (half + 1) * 512)
                    pl_ps = psC.tile([128, 512], F32, tag="bq")
                    nc.tensor.matmul(pl_ps, lhsT=q16q[:, bt * 128:(bt + 1) * 128],
                                     rhs=centn[:, sl])
                    nc.scalar.copy(plog[:, sl], pl_ps)
                mx8 = poolC.tile([128, 8], F32, tag="mx8")
                ix8 = poolC.tile([128, 8], mybir.dt.uint32, tag="ix8")
                nc.vector.max_with_indices(mx8, ix8, plog)
                nc.vector.tensor_copy(pseudo_i[:, bt:bt + 1], ix8[:, 0:1])
            # export pseudo (as f32) for host-side exact mask counts
            nc.vector.tensor_copy(stats_sb[:, 16:24], pseudo_i)

        # =========== Phase D: the B x KS similarity / mae / mask loop =======
        acc_ms = persist.tile([128, NBT * NCH], F32)   # masked mae sums
        acc_mt = persist.tile([128, NBT * NCH], F32)   # total mae sums
        with tc.tile_pool(name="psD", bufs=2, space="PSUM") as psD, \
             tc.tile_pool(name="psE", bufs=1, space="PSUM") as psE, \
             tc.tile_pool(name="poolD", bufs=3) as poolD, \
             tc.tile_pool(name="outD", bufs=4) as outD:
            for ch in range(NCH):
                c0 = ch * CH
                qt = poolD.tile([128, CH], F16, tag="qt")
                nc.sync.dma_start(out=qt, in_=qslT[:, c0:c0 + CH])
                ct = poolD.tile([128, CH], F16, tag="ct")
                nc.sync.dma_start(out=ct, in_=cslT[:, c0:c0 + CH])
                labb = poolD.tile([128, CH], F32, tag="labb")
                lab_sl = labs[c0:c0 + CH]
                nc.gpsimd.dma_start(
                    out=labb,
                    in_=bass.AP(tensor=lab_sl.tensor, offset=lab_sl.offset,
                                ap=[[0, 128]] + list(lab_sl.ap)))
                for bt in range(NBT):
                    bsl = slice(bt * 128, (bt + 1) * 128)
                    ci = bt * NCH + ch
                    sq_ps = psD.tile([128, CH], F32, tag="sq")
                    nc.tensor.matmul(sq_ps, lhsT=q16q[:, bsl], rhs=qt)
                    lgch = outD.tile([128, CH], F32, tag="lgch")
                    nc.vector.tensor_copy(lgch, sq_ps)
                    nc.sync.dma_start(out=lneg[bsl, c0:c0 + CH], in_=lgch)
                    sc_ps = psD.tile([128, CH], F32, tag="sc")
                    nc.tensor.matmul(sc_ps, lhsT=q16c[:, bsl], rhs=ct)
                    mae = poolD.tile([128, CH], F16, tag="mae")
                    nc.scalar.activation(mae, sc_ps, AF.Sqrt,
                                         bias=maebias[:, 0:1], scale=1.0,
                                         accum_out=acc_mt[:, ci:ci + 1])
                    mask = poolD.tile([128, CH], F16, tag="mask")
                    nc.gpsimd.tensor_scalar(mask, labb, pseudo_i[:, bt:bt + 1],
                                            None, OP.is_equal)
                    scr = poolD.tile([128, CH], F16, tag="scr")
                    nc.vector.tensor_mul(scr, mae, mask)
                    nc.vector.tensor_reduce(acc_ms[:, ci:ci + 1], scr,
                                            axis=AX.X, op=OP.add)

            # =========== Phase E: classification head (own rows) ============
            wc_sb = persist.tile([128, ET, NCLS], F16)
            nc.sync.dma_start(out=wc_sb, in_=wc.rearrange("(t p) c -> p t c", p=128))
            bcls_bc = persist.tile([128, NCLS], F32)
            nc.gpsimd.dma_start(
                out=bcls_bc,
                in_=bass.AP(tensor=bcls.ap().tensor, offset=0,
                            ap=[[0, 128]] + list(bcls.ap().ap)))
            clab_sb = persist.tile([128, 1], F32)
            nc.sync.dma_start(out=clab_sb, in_=clab.rearrange("(p o) -> p o", o=1))
            cls_ps = psE.tile([128, NCLS], F32, tag="cls")
            for et in range(ET):
                nc.tensor.matmul(cls_ps, lhsT=hq_sb[:, et, :], rhs=wc_sb[:, et, :],
                                 start=(et == 0), stop=(et == ET - 1))
            coarse = poolD.tile([128, NCLS], F32, tag="coarse")
            nc.vector.tensor_add(coarse, cls_ps, bcls_bc)
            mxc = poolD.tile([128, 1], F32, tag="mxc")
            nc.vector.tensor_reduce(mxc, coarse, axis=AX.X, op=OP.max)
            mxn = poolD.tile([128, 1], F32, tag="mxn")
            nc.vector.tensor_scalar_mul(mxn, mxc, -1.0)
            es = poolD.tile([128, NCLS], F16, tag="es")
            sume = poolD.tile([128, 1], F32, tag="sume")
            nc.scalar.activation(es, coarse, AF.Exp, bias=mxn[:, 0:1], scale=1.0,
                                 accum_out=sume)
            lse = poolD.tile([128, 1], F32, tag="lse")
            nc.scalar.activation(lse, sume, AF.Ln, bias=0.0, scale=1.0)
            ohc = poolD.tile([128, NCLS], F32, tag="ohc")
            nc.vector.tensor_scalar(ohc, iota_i[:, 0:NCLS], clab_sb, None,
                                    OP.is_equal)
            scrc = poolD.tile([128, NCLS], F32, tag="ohscr")
            picked = poolD.tile([128, 1], F32, tag="picked")
            nc.vector.tensor_mul(scrc, coarse, ohc)
            nc.vector.tensor_reduce(picked, scrc, axis=AX.X, op=OP.add)
            t1 = poolD.tile([128, 1], F32, tag="t1")
            nc.vector.tensor_sub(t1, picked, mxc)
            nc.vector.tensor_sub(stats_sb[:, 24:25], t1, lse)

            # =========== Final: fold chunk accumulators, write stats ========
            for bt in range(NBT):
                sl = slice(bt * NCH, (bt + 1) * NCH)
                nc.vector.tensor_reduce(stats_sb[:, bt:bt + 1], acc_ms[:, sl],
                                        axis=AX.X, op=OP.add)
                nc.vector.tensor_reduce(stats_sb[:, 8 + bt:9 + bt], acc_mt[:, sl],
                                        axis=AX.X, op=OP.add)
            nc.sync.dma_start(out=stats.ap(), in_=stats_sb)

    nc.compile()
    return nc


_NC_CACHE = None


def _get_nc():
    global _NC_CACHE
    if _NC_CACHE is None:
        _NC_CACHE = build_kernel()
    return _NC_CACHE


def kernel(h_q, h_k, W_cls, b_cls, W_p1, b_p1, W_p2, b_p2,
           Wk_p1, bk_p1, Wk_p2, bk_p2, queue_emb, queue_emb_copy,
           info_label, coarse_labs):
    nc = _get_nc()
    f16 = np.float16
    f32 = np.float32
    ca = np.ascontiguousarray

    h_q = np.asarray(h_q, f32)
    h_k = np.asarray(h_k, f32)
    queue_emb = np.asarray(queue_emb, f32)
    queue_emb_copy = np.asarray(queue_emb_copy, f32)
    info_label = np.asarray(info_label).astype(np.int32)
    coarse_labs = np.asarray(coarse_labs).astype(np.int32)

    w1 = ca(np.asarray(W_p1, f16))
    wk1 = ca(np.asarray(Wk_p1, f16))
    w2 = ca(np.asarray(W_p2, f16))
    wk2 = ca(np.asarray(Wk_p2, f16))
    wc = ca(np.asarray(W_cls, f16))
    bp1 = ca(np.asarray(b_p1, f32))
    bkp1 = ca(np.asarray(bk_p1, f32))
    bp2 = ca(np.asarray(b_p2, f32))
    bkp2 = ca(np.asarray(bk_p2, f32))
    bcv = ca(np.asarray(b_cls, f32))
    queueT = ca(queue_emb.T.astype(f16))        # [D, K]
    copyT = ca(queue_emb_copy.T.astype(f16))    # [D, K]

    in_maps = []
    for c in range(NCORES):
        bs = slice(c * BS, (c + 1) * BS)
        ks = slice(c * KS, (c + 1) * KS)
        in_maps.append({
            "hqT": ca(h_q[bs].T.astype(f16)),
            "hkT": ca(h_k[bs].T.astype(f16)),
            "w1": w1, "wk1": wk1, "w2": w2, "wk2": wk2, "wc": wc,
            "bp1": bp1, "bkp1": bkp1, "bp2": bp2, "bkp2": bkp2, "bcls": bcv,
            "qslT": ca(queueT[:, ks]),
            "cslT": ca(copyT[:, ks]),
            "cnat": ca(queue_emb_copy[ks].astype(f16)),
            "labs": ca(info_label[ks].astype(f32)),
            "clab": ca(coarse_labs[bs].astype(f32)),
        })

    res = run_bass_kernel_spmd(nc, in_maps, list(range(NCORES)))
    results = res.results

    # ---- host-side gather / final scalar math ----
    logits = np.empty((B, 1 + K), f32)
    for c in range(NCORES):
        logits[:, 1 + c * KS: 1 + (c + 1) * KS] = results[c]["lneg"]
    stats = [results[c]["stats"] for c in range(NCORES)]
    for c in range(NCORES):
        logits[c * BS:(c + 1) * BS, 0] = stats[c][:, 25]

    # masked mae sums / total mae sums: partial over each core's K slice
    msum = np.zeros((B,), np.float64)
    mtot = np.zeros((B,), np.float64)
    for c in range(NCORES):
        msum += stats[c][:, 0:8].T.reshape(B).astype(np.float64)
        mtot += stats[c][:, 8:16].T.reshape(B).astype(np.float64)
    # pseudo labels (identical on every core; take core 0)
    pseudo = stats[0][:, 16:24].T.reshape(B).astype(np.int64)
    class_counts = np.bincount(info_label, minlength=NI)
    cnt = class_counts[pseudo].astype(np.float64)

    eps = 1e-6
    min_e = np.mean(msum / (cnt + eps))
    avg_inter = np.mean((mtot - msum) / (K - cnt + eps))
    dino_loss = np.float32(min_e + (2.0 - avg_inter))

    logp = np.concatenate([stats[c][:, 24] for c in range(NCORES)])
    cls_loss = np.float32(-logp.mean())

    labels = np.zeros((B,), np.int32)
    return logits, labels, dino_loss, cls_loss


# revision 8
# speedup vs baseline: 2.5419x; 2.5419x over previous
"""MoCo forward kernel for 8 Trainium2 NeuronCores.

Sharding:
  - Projections (h->q, h->k), l_pos, cls head: data-parallel over batch
    (128 rows per core), followed by an AllGather of raw q vectors.
  - Per-info-class centroid sums: sharded over the queue dim K
    (8192 rows per core) via a one-hot matmul, combined with an AllReduce.
  - The big B x K similarity / mask / MAE phase: sharded over K columns
    (each core computes all 1024 rows against its 8192 queue columns),
    which avoids replicating the 2x 32MB queue reads on every core.

All matmuls run in fp16 (same PE rate as bf16, 4x the mantissa) with fp32
PSUM accumulation; l2 normalization factors are folded into pre-scaled
copies of q so the inner loop is pure matmul + one ACT sqrt + one DVE
copy + one DVE masked-reduce + one GPSIMD compare per tile.
"""

import sys

sys.path.insert(0, '/opt/trn_rl_repo')

from contextlib import ExitStack

import numpy as np

import concourse.bass as bass
import concourse.mybir as mybir
import concourse.tile as tile
from concourse import bacc
from concourse.bass_utils import run_bass_kernel_spmd

NCORES = 8
B, E, D, K = 1024, 2048, 128, 65536
NI, NCLS = 1024, 100
T = 0.07
BS = B // NCORES          # 128 batch rows per core
KS = K // NCORES          # 8192 queue rows/cols per core
CH = 512                  # chunk width in the K loop
NCH = KS // CH            # 16 chunks per core
NBT = B // 128            # 8 batch tiles of 128
ET = E // 128             # 16 contraction tiles over EMBED

F16 = mybir.dt.float16
BF16 = mybir.dt.bfloat16
F32 = mybir.dt.float32
I32 = mybir.dt.int32
AX = mybir.AxisListType
OP = mybir.AluOpType
AF = mybir.ActivationFunctionType


def build_kernel():
    nc = bacc.Bacc('TRN2', target_bir_lowering=False, debug=False,
                   num_devices=NCORES)

    # ---- DRAM I/O ----
    hqT = nc.dram_tensor("hqT", [E, BS], F16, kind="ExternalInput")
    hkT = nc.dram_tensor("hkT", [E, BS], F16, kind="ExternalInput")
    w1 = nc.dram_tensor("w1", [E, E], F16, kind="ExternalInput")
    wk1 = nc.dram_tensor("wk1", [E, E], F16, kind="ExternalInput")
    w2 = nc.dram_tensor("w2", [E, D], F16, kind="ExternalInput")
    wk2 = nc.dram_tensor("wk2", [E, D], F16, kind="ExternalInput")
    wc = nc.dram_tensor("wc", [E, NCLS], F16, kind="ExternalInput")
    bp1 = nc.dram_tensor("bp1", [E], F32, kind="ExternalInput")
    bkp1 = nc.dram_tensor("bkp1", [E], F32, kind="ExternalInput")
    bp2 = nc.dram_tensor("bp2", [D], F32, kind="ExternalInput")
    bkp2 = nc.dram_tensor("bkp2", [D], F32, kind="ExternalInput")
    bcls = nc.dram_tensor("bcls", [NCLS], F32, kind="ExternalInput")
    qslT = nc.dram_tensor("qslT", [D, KS], F16, kind="ExternalInput")
    cslT = nc.dram_tensor("cslT", [D, KS], F16, kind="ExternalInput")
    cnat = nc.dram_tensor("cnat", [KS, D], BF16, kind="ExternalInput")
    labs = nc.dram_tensor("labs", [KS], F32, kind="ExternalInput")
    clab = nc.dram_tensor("clab", [BS], F32, kind="ExternalInput")

    lneg = nc.dram_tensor("lneg", [B, KS], F32, kind="ExternalOutput")
    stats = nc.dram_tensor("stats", [BS, 32], F32, kind="ExternalOutput")

    groups = [list(range(NCORES))]

    with tile.TileContext(nc) as tc, ExitStack() as ctx:
        consts = ctx.enter_context(tc.tile_pool(name="consts", bufs=1))
        persist = ctx.enter_context(tc.tile_pool(name="persist", bufs=1))
        dram = ctx.enter_context(tc.tile_pool(name="dram", bufs=1, space="DRAM"))

        # constants
        iota_i = consts.tile([128, NI], F32)
        nc.gpsimd.iota(iota_i, pattern=[[1, NI]], base=0, channel_multiplier=0,
                       allow_small_or_imprecise_dtypes=True)
        ones_col = consts.tile([128, 1], F32)
        nc.vector.memset(ones_col, 1.0)
        ones_row = consts.tile([1, 128], F32)
        nc.vector.memset(ones_row, 1.0)

        stats_sb = persist.tile([128, 32], F32)
        nc.vector.memset(stats_sb, 0.0)
        eps30 = consts.tile([1, 1], F32)
        nc.vector.memset(eps30, 1e-30)
        maebias = consts.tile([128, 1], F32)
        nc.vector.memset(maebias, 2.0 + 1e-6)

        # =========== Phase A: centroid partial sums over own K slice =========
        # cent_ps[d, i] += sum_k copy[k, d] * onehot[k, i]
        cent_sum = persist.tile([128, NI], F32)
        with tc.tile_pool(name="psA", bufs=1, space="PSUM") as psA, \
             tc.tile_pool(name="poolA", bufs=3) as poolA:
            cent_ps0 = psA.tile([128, 512], F32, tag="c0")
            cent_ps1 = psA.tile([128, 512], F32, tag="c1")
            nkt = KS // 128  # 64
            for kt in range(nkt):
                cpt = poolA.tile([128, D], BF16, tag="cpt")
                nc.sync.dma_start(out=cpt, in_=cnat[kt * 128:(kt + 1) * 128, :])
                labc = poolA.tile([128, 1], F32, tag="labc")
                nc.sync.dma_start(
                    out=labc,
                    in_=labs[kt * 128:(kt + 1) * 128].rearrange("(p o) -> p o", o=1))
                oh = poolA.tile([128, NI], BF16, tag="oh")
                nc.vector.tensor_scalar(oh, iota_i, labc, None, OP.is_equal)
                nc.tensor.matmul(cent_ps0, lhsT=cpt, rhs=oh[:, 0:512],
                                 start=(kt == 0), stop=(kt == nkt - 1))
                nc.tensor.matmul(cent_ps1, lhsT=cpt, rhs=oh[:, 512:NI],
                                 start=(kt == 0), stop=(kt == nkt - 1))
            nc.scalar.copy(cent_sum[:, 0:512], cent_ps0)
            nc.scalar.copy(cent_sum[:, 512:NI], cent_ps1)

        cent_in = dram.tile([128, NI], F32)
        cent_out = dram.tile([128, NI], F32)
        nc.sync.dma_start(out=cent_in, in_=cent_sum)
        nc.gpsimd.collective_compute(
            "AllReduce", OP.add, replica_groups=groups,
            ins=[cent_in.opt()], outs=[cent_out.opt()])
        centall = persist.tile([128, NI], F32)
        nc.sync.dma_start(out=centall, in_=cent_out)

        # =========== Phase B: projections (own 128 batch rows) ==============
        hq_sb = persist.tile([128, ET, BS], F16)
        nc.sync.dma_start(out=hq_sb, in_=hqT.rearrange("(t p) b -> p t b", p=128))
        hk_sb = persist.tile([128, ET, BS], F16)
        nc.sync.dma_start(out=hk_sb, in_=hkT.rearrange("(t p) b -> p t b", p=128))
        w2_sb = persist.tile([128, ET, D], F16)
        nc.sync.dma_start(out=w2_sb, in_=w2.rearrange("(t p) d -> p t d", p=128))
        wk2_sb = persist.tile([128, ET, D], F16)
        nc.sync.dma_start(out=wk2_sb, in_=wk2.rearrange("(t p) d -> p t d", p=128))
        bp1_sb = persist.tile([128, ET], F32)
        nc.sync.dma_start(out=bp1_sb, in_=bp1.rearrange("(t p) -> p t", p=128))
        bkp1_sb = persist.tile([128, ET], F32)
        nc.sync.dma_start(out=bkp1_sb, in_=bkp1.rearrange("(t p) -> p t", p=128))
        bp2_sb = persist.tile([128, 1], F32)
        nc.sync.dma_start(out=bp2_sb, in_=bp2.rearrange("(p o) -> p o", o=1))
        bkp2_sb = persist.tile([128, 1], F32)
        nc.sync.dma_start(out=bkp2_sb, in_=bkp2.rearrange("(p o) -> p o", o=1))

        qraw = persist.tile([128, BS], F32)   # [d, b] fp32, own shard
        kraw = persist.tile([128, BS], F32)

        with tc.tile_pool(name="psB", bufs=1, space="PSUM") as psB, \
             tc.tile_pool(name="poolB", bufs=3) as poolB, \
             tc.tile_pool(name="z1", bufs=1) as z1pool:
            ps_small = psB.tile([128, 16], F32, tag="small")
            for branch in range(2):
                wsrc = w1 if branch == 0 else wk1
                hsb = hq_sb if branch == 0 else hk_sb
                b1sb = bp1_sb if branch == 0 else bkp1_sb
                w2sb = w2_sb if branch == 0 else wk2_sb
                b2sb = bp2_sb if branch == 0 else bkp2_sb
                zout = qraw if branch == 0 else kraw
                z1_sb = z1pool.tile([128, ET, BS], F16, tag=f"z1_{branch}")
                for ot in range(ET):
                    wcol = poolB.tile([128, ET, 128], F16, tag="wcol")
                    nc.sync.dma_start(
                        out=wcol,
                        in_=wsrc[:, ot * 128:(ot + 1) * 128]
                        .rearrange("(t p) m -> p t m", p=128))
                    zps = psB.tile([128, BS], F32, tag="zps")
                    for et in range(ET):
                        nc.tensor.matmul(zps, lhsT=wcol[:, et, :],
                                         rhs=hsb[:, et, :],
                                         start=(et == 0), stop=(et == ET - 1))
                    nc.scalar.activation(z1_sb[:, ot, :], zps, AF.Relu,
                                         bias=b1sb[:, ot:ot + 1], scale=1.0)
                qps = psB.tile([128, BS], F32, tag="zps")
                for et in range(ET):
                    nc.tensor.matmul(qps, lhsT=w2sb[:, et, :], rhs=z1_sb[:, et, :],
                                     start=(et == 0), stop=(et == ET - 1))
                nc.scalar.activation(zout, qps, AF.Identity,
                                     bias=b2sb[:, 0:1], scale=1.0)

            # own-shard norms and l_pos
            sq = poolB.tile([128, BS], F32, tag="sqloc")
            nc.vector.tensor_mul(sq, qraw, qraw)
            nc.tensor.matmul(ps_small[:, 8:9], lhsT=sq, rhs=ones_col)
            sk = poolB.tile([128, BS], F32, tag="sqloc")
            nc.vector.tensor_mul(sk, kraw, kraw)
            nc.tensor.matmul(ps_small[:, 9:10], lhsT=sk, rhs=ones_col)
            pq = poolB.tile([128, BS], F32, tag="sqloc")
            nc.vector.tensor_mul(pq, qraw, kraw)
            nc.tensor.matmul(ps_small[:, 10:11], lhsT=pq, rhs=ones_col)
            nrm2 = persist.tile([128, 4], F32)
            nc.scalar.activation(nrm2[:, 0:2], ps_small[:, 8:10], AF.Sqrt,
                                 bias=0.0, scale=1.0)
            rloc = persist.tile([128, 2], F32)
            nc.vector.reciprocal(rloc, nrm2[:, 0:2])
            # l_pos = (q.k) * rnq * rnk / T  -> stats col 25
            lposv = persist.tile([128, 1], F32)
            nc.vector.tensor_scalar(lposv, ps_small[:, 10:11],
                                    rloc[:, 0:1], rloc[:, 1:2], OP.mult, OP.mult)
            nc.vector.tensor_scalar_mul(stats_sb[:, 25:26], lposv, 1.0 / T)

        # AllGather raw q across cores -> [d, B] on every core
        ag_in = dram.tile([128, BS], F32)
        ag_out = dram.tile([NCORES, 128, BS], F32)
        nc.sync.dma_start(out=ag_in, in_=qraw)
        nc.gpsimd.collective_compute(
            "AllGather", OP.bypass, replica_groups=groups,
            ins=[ag_in.opt()], outs=[ag_out.opt()])
        qall = persist.tile([128, B], F32)
        for c in range(NCORES):
            nc.sync.dma_start(out=qall[:, c * BS:(c + 1) * BS], in_=ag_out[c])

        # global norms (row layout) and pre-scaled fp16 copies of q
        q16q = persist.tile([128, B], F16)   # q * rnq / T      (logits)
        q16c = persist.tile([128, B], F16)   # q * (-2 rnq)     (mae)
        with tc.tile_pool(name="psC", bufs=2, space="PSUM") as psC, \
             tc.tile_pool(name="poolC", bufs=2) as poolC:
            sqall = poolC.tile([128, B], F32, tag="sqall")
            nc.vector.tensor_mul(sqall, qall, qall)
            nrow_ps = psC.tile([1, B], F32, tag="nrow")
            nc.tensor.matmul(nrow_ps[:, 0:512], lhsT=ones_col, rhs=sqall[:, 0:512])
            nc.tensor.matmul(nrow_ps[:, 512:B], lhsT=ones_col, rhs=sqall[:, 512:B])
            nrow = poolC.tile([1, B], F32, tag="nrowsb")
            nc.scalar.activation(nrow[:, 0:512], nrow_ps[:, 0:512], AF.Sqrt,
                                 bias=0.0, scale=1.0)
            nc.scalar.activation(nrow[:, 512:B], nrow_ps[:, 512:B], AF.Sqrt,
                                 bias=0.0, scale=1.0)
            rrow = poolC.tile([1, B], F32, tag="rrow")
            nc.vector.reciprocal(rrow, nrow)
            rq_row = poolC.tile([1, B], F32, tag="rqrow")
            nc.vector.tensor_scalar_mul(rq_row, rrow, 1.0 / T)
            rc_row = poolC.tile([1, B], F32, tag="rcrow")
            nc.vector.tensor_scalar_mul(rc_row, rrow, -2.0)
            for half in range(2):
                sl = slice(half * 512, (half + 1) * 512)
                bq_ps = psC.tile([128, 512], F32, tag="bq")
                nc.tensor.matmul(bq_ps, lhsT=ones_row, rhs=rq_row[:, sl])
                nc.vector.tensor_mul(q16q[:, sl], qall[:, sl], bq_ps)
                bc_ps = psC.tile([128, 512], F32, tag="bq")
                nc.tensor.matmul(bc_ps, lhsT=ones_row, rhs=rc_row[:, sl])
                nc.vector.tensor_mul(q16c[:, sl], qall[:, sl], bc_ps)

            # ========= Phase C: centroid normalize + pseudo labels ==========
            sqc = poolC.tile([128, NI], F32, tag="sqall")
            nc.vector.tensor_mul(sqc, centall, centall)
            cn_ps = psC.tile([1, NI], F32, tag="nrow")
            nc.tensor.matmul(cn_ps[:, 0:512], lhsT=ones_col, rhs=sqc[:, 0:512])
            nc.tensor.matmul(cn_ps[:, 512:NI], lhsT=ones_col, rhs=sqc[:, 512:NI])
            cnn = poolC.tile([1, NI], F32, tag="nrowsb")
            nc.scalar.activation(cnn[:, 0:512], cn_ps[:, 0:512], AF.Sqrt,
                                 bias=eps30[0:1, 0:1], scale=1.0)
            nc.scalar.activation(cnn[:, 512:NI], cn_ps[:, 512:NI], AF.Sqrt,
                                 bias=eps30[0:1, 0:1], scale=1.0)
            crn = poolC.tile([1, NI], F32, tag="rrow")
            nc.vector.reciprocal(crn, cnn)
            centn = persist.tile([128, NI], F16)
            for half in range(2):
                sl = slice(half * 512, (half + 1) * 512)
                cb_ps = psC.tile([128, 512], F32, tag="bq")
                nc.tensor.matmul(cb_ps, lhsT=ones_row, rhs=crn[:, sl])
                nc.vector.tensor_mul(centn[:, sl], centall[:, sl], cb_ps)

            pseudo_i = persist.tile([128, NBT], F32)
            for bt in range(NBT):
                plog = poolC.tile([128, NI], F32, tag="plog")
                for half in range(2):
                    sl = slice(half * 512, (half + 1) * 512)
                    pl_ps = psC.tile([128, 512], F32, tag="bq")
                    nc.tensor.matmul(pl_ps, lhsT=q16q[:, bt * 128:(bt + 1) * 128],
                                     rhs=centn[:, sl])
                    nc.scalar.copy(plog[:, sl], pl_ps)
                mx8 = poolC.tile([128, 8], F32, tag="mx8")
                ix8 = poolC.tile([128, 8], mybir.dt.uint32, tag="ix8")
                nc.vector.max_with_indices(mx8, ix8, plog)
                nc.vector.tensor_copy(pseudo_i[:, bt:bt + 1], ix8[:, 0:1])
            # export pseudo (as f32) for host-side exact mask counts
            nc.vector.tensor_copy(stats_sb[:, 16:24], pseudo_i)

        # =========== Phase D: the B x KS similarity / mae / mask loop =======
        acc_ms = persist.tile([128, NBT * NCH], F32)   # masked mae sums
        acc_mt = persist.tile([128, NBT * NCH], F32)   # total mae sums
        with tc.tile_pool(name="psD", bufs=2, space="PSUM") as psD, \
             tc.tile_pool(name="psE", bufs=1, space="PSUM") as psE, \
             tc.tile_pool(name="poolD", bufs=3) as poolD, \
             tc.tile_pool(name="outD", bufs=4) as outD:
            for ch in range(NCH):
                c0 = ch * CH
                qt = poolD.tile([128, CH], F16, tag="qt")
                nc.sync.dma_start(out=qt, in_=qslT[:, c0:c0 + CH])
                ct = poolD.tile([128, CH], F16, tag="ct")
                nc.sync.dma_start(out=ct, in_=cslT[:, c0:c0 + CH])
                labb = poolD.tile([128, CH], F32, tag="labb")
                lab_sl = labs[c0:c0 + CH]
                nc.gpsimd.dma_start(
                    out=labb,
                    in_=bass.AP(tensor=lab_sl.tensor, offset=lab_sl.offset,
                                ap=[[0, 128]] + list(lab_sl.ap)))
                for bt in range(NBT):
                    bsl = slice(bt * 128, (bt + 1) * 128)
                    ci = bt * NCH + ch
                    sq_ps = psD.tile([128, CH], F32, tag="sq")
                    nc.tensor.matmul(sq_ps, lhsT=q16q[:, bsl], rhs=qt)
                    lgch = outD.tile([128, CH], F32, tag="lgch")
                    nc.scalar.copy(lgch, sq_ps)
                    nc.sync.dma_start(out=lneg[bsl, c0:c0 + CH], in_=lgch)
                    sc_ps = psD.tile([128, CH], F32, tag="sc")
                    nc.tensor.matmul(sc_ps, lhsT=q16c[:, bsl], rhs=ct)
                    mae = poolD.tile([128, CH], BF16, tag="mae")
                    nc.scalar.activation(mae, sc_ps, AF.Sqrt,
                                         bias=maebias[:, 0:1], scale=1.0,
                                         accum_out=acc_mt[:, ci:ci + 1])
                    mask = poolD.tile([128, CH], BF16, tag="mask")
                    nc.vector.tensor_scalar(mask, labb, pseudo_i[:, bt:bt + 1],
                                            None, OP.is_equal)
                    scr = poolD.tile([128, CH], BF16, tag="scr")
                    nc.vector.tensor_mul(scr, mae, mask)
                    nc.vector.tensor_reduce(acc_ms[:, ci:ci + 1], scr,
                                            axis=AX.X, op=OP.add)

            # =========== Phase E: classification head (own rows) ============
            wc_sb = persist.tile([128, ET, NCLS], F16)
            nc.sync.dma_start(out=wc_sb, in_=wc.rearrange("(t p) c -> p t c", p=128))
            bcls_bc = persist.tile([128, NCLS], F32)
            nc.gpsimd.dma_start(
                out=bcls_bc,
                in_=bass.AP(tensor=bcls.ap().tensor, offset=0,
                            ap=[[0, 128]] + list(bcls.ap().ap)))
            clab_sb = persist.tile([128, 1], F32)
            nc.sync.dma_start(out=clab_sb, in_=clab.rearrange("(p o) -> p o", o=1))
            cls_ps = psE.tile([128, NCLS], F32, tag="cls")
            for et in range(ET):
                nc.tensor.matmul(cls_ps, lhsT=hq_sb[:, et, :], rhs=wc_sb[:, et, :],
                                 start=(et == 0), stop=(et == ET - 1))
            coarse = poolD.tile([128, NCLS], F32, tag="coarse")
            nc.vector.tensor_add(coarse, cls_ps, bcls_bc)
            mxc = poolD.tile([128, 1], F32, tag="mxc")
            nc.vector.tensor_reduce(mxc, coarse, axis=AX.X, op=OP.max)
            mxn = poolD.tile([128, 1], F32, tag="mxn")
            nc.vector.tensor_scalar_mul(mxn, mxc, -1.0)
            es = poolD.tile([128, NCLS], BF16, tag="es")
            sume = poolD.tile([128, 1], F32, tag="sume")
            nc.scalar.activation(es, coarse, AF.Exp, bias=mxn[:, 0:1], scale=1.0,
                                 accum_out=sume)
            lse = poolD.tile([128, 1], F32, tag="lse")
            nc.scalar.activation(lse, sume, AF.Ln, bias=0.0, scale=1.0)
            ohc = poolD.tile([128, NCLS], F32, tag="ohc")
            nc.vector.tensor_scalar(ohc, iota_i[:, 0:NCLS], clab_sb, None,
                                    OP.is_equal)
            scrc = poolD.tile([128, NCLS], F32, tag="ohscr")
            picked = poolD.tile([128, 1], F32, tag="picked")
            nc.vector.tensor_mul(scrc, coarse, ohc)
            nc.vector.tensor_reduce(picked, scrc, axis=AX.X, op=OP.add)
            t1 = poolD.tile([128, 1], F32, tag="t1")
            nc.vector.tensor_sub(t1, picked, mxc)
            nc.vector.tensor_sub(stats_sb[:, 24:25], t1, lse)

            # =========== Final: fold chunk accumulators, write stats ========
            for bt in range(NBT):
                sl = slice(bt * NCH, (bt + 1) * NCH)
                nc.vector.tensor_reduce(stats_sb[:, bt:bt + 1], acc_ms[:, sl],
                                        axis=AX.X, op=OP.add)
                nc.vector.tensor_reduce(stats_sb[:, 8 + bt:9 + bt], acc_mt[:, sl],
                                        axis=AX.X, op=OP.add)
            nc.sync.dma_start(out=stats.ap(), in_=stats_sb)

    nc.compile()
    return nc


_NC_CACHE = None


def _get_nc():
    global _NC_CACHE
    if _NC_CACHE is None:
        _NC_CACHE = build_kernel()
    return _NC_CACHE


def kernel(h_q, h_k, W_cls, b_cls, W_p1, b_p1, W_p2, b_p2,
           Wk_p1, bk_p1, Wk_p2, bk_p2, queue_emb, queue_emb_copy,
           info_label, coarse_labs):
    nc = _get_nc()
    import ml_dtypes
    bf16_t = ml_dtypes.bfloat16
    f16 = np.float16
    f32 = np.float32
    ca = np.ascontiguousarray

    h_q = np.asarray(h_q, f32)
    h_k = np.asarray(h_k, f32)
    queue_emb = np.asarray(queue_emb, f32)
    queue_emb_copy = np.asarray(queue_emb_copy, f32)
    info_label = np.asarray(info_label).astype(np.int32)
    coarse_labs = np.asarray(coarse_labs).astype(np.int32)

    w1 = ca(np.asarray(W_p1, f16))
    wk1 = ca(np.asarray(Wk_p1, f16))
    w2 = ca(np.asarray(W_p2, f16))
    wk2 = ca(np.asarray(Wk_p2, f16))
    wc = ca(np.asarray(W_cls, f16))
    bp1 = ca(np.asarray(b_p1, f32))
    bkp1 = ca(np.asarray(bk_p1, f32))
    bp2 = ca(np.asarray(b_p2, f32))
    bkp2 = ca(np.asarray(bk_p2, f32))
    bcv = ca(np.asarray(b_cls, f32))
    queueT = ca(queue_emb.T.astype(f16))        # [D, K]
    copyT = ca(queue_emb_copy.T.astype(f16))    # [D, K]

    in_maps = []
    for c in range(NCORES):
        bs = slice(c * BS, (c + 1) * BS)
        ks = slice(c * KS, (c + 1) * KS)
        in_maps.append({
            "hqT": ca(h_q[bs].T.astype(f16)),
            "hkT": ca(h_k[bs].T.astype(f16)),
            "w1": w1, "wk1": wk1, "w2": w2, "wk2": wk2, "wc": wc,
            "bp1": bp1, "bkp1": bkp1, "bp2": bp2, "bkp2": bkp2, "bcls": bcv,
            "qslT": ca(queueT[:, ks]),
            "cslT": ca(copyT[:, ks]),
            "cnat": ca(queue_emb_copy[ks].astype(bf16_t)),
            "labs": ca(info_label[ks].astype(f32)),
            "clab": ca(coarse_labs[bs].astype(f32)),
        })

    res = run_bass_kernel_spmd(nc, in_maps, list(range(NCORES)))
    results = res.results

    # ---- host-side gather / final scalar math ----
    logits = np.empty((B, 1 + K), f32)
    for c in range(NCORES):
        logits[:, 1 + c * KS: 1 + (c + 1) * KS] = results[c]["lneg"]
    stats = [results[c]["stats"] for c in range(NCORES)]
    for c in range(NCORES):
        logits[c * BS:(c + 1) * BS, 0] = stats[c][:, 25]

    # masked mae sums / total mae sums: partial over each core's K slice
    msum = np.zeros((B,), np.float64)
    mtot = np.zeros((B,), np.float64)
    for c in range(NCORES):
        msum += stats[c][:, 0:8].T.reshape(B).astype(np.float64)
        mtot += stats[c][:, 8:16].T.reshape(B).astype(np.float64)
    # pseudo labels (identical on every core; take core 0)
    pseudo = stats[0][:, 16:24].T.reshape(B).astype(np.int64)
    class_counts = np.bincount(info_label, minlength=NI)
    cnt = class_counts[pseudo].astype(np.float64)

    eps = 1e-6
    min_e = np.mean(msum / (cnt + eps))
    avg_inter = np.mean((mtot - msum) / (K - cnt + eps))
    dino_loss = np.float32(min_e + (2.0 - avg_inter))

    logp = np.concatenate([stats[c][:, 24] for c in range(NCORES)])
    cls_loss = np.float32(-logp.mean())

    labels = np.zeros((B,), np.int32)
    return logits, labels, dino_loss, cls_loss


# revision 9
# speedup vs baseline: 2.7859x; 1.0960x over previous
"""MoCo forward kernel for 8 Trainium2 NeuronCores.

Sharding:
  - Projections (h->q, h->k), l_pos, cls head: data-parallel over batch
    (128 rows per core), followed by an AllGather of raw q vectors.
  - Per-info-class centroid sums: sharded over the queue dim K
    (8192 rows per core) via a one-hot matmul, combined with an AllReduce.
  - The big B x K similarity / mask / MAE phase: sharded over K columns
    (each core computes all 1024 rows against its 8192 queue columns),
    which avoids replicating the 2x 32MB queue reads on every core.

All matmuls run in fp16 (same PE rate as bf16, 4x the mantissa) with fp32
PSUM accumulation; l2 normalization factors are folded into pre-scaled
copies of q so the inner loop is pure matmul + one ACT sqrt + one DVE
copy + one DVE masked-reduce + one GPSIMD compare per tile.
"""

import sys

sys.path.insert(0, '/opt/trn_rl_repo')

from contextlib import ExitStack

import numpy as np

import concourse.bass as bass
import concourse.mybir as mybir
import concourse.tile as tile
from concourse import bacc
from concourse.bass_utils import run_bass_kernel_spmd

NCORES = 8
B, E, D, K = 1024, 2048, 128, 65536
NI, NCLS = 1024, 100
T = 0.07
BS = B // NCORES          # 128 batch rows per core
KS = K // NCORES          # 8192 queue rows/cols per core
CH = 1024                 # chunk width in the K loop
NCH = KS // CH            # 16 chunks per core
NBT = B // 128            # 8 batch tiles of 128
ET = E // 128             # 16 contraction tiles over EMBED

F16 = mybir.dt.float16
BF16 = mybir.dt.bfloat16
F32 = mybir.dt.float32
I32 = mybir.dt.int32
AX = mybir.AxisListType
OP = mybir.AluOpType
AF = mybir.ActivationFunctionType


def build_kernel():
    nc = bacc.Bacc('TRN2', target_bir_lowering=False, debug=False,
                   num_devices=NCORES)

    # ---- DRAM I/O ----
    hqT = nc.dram_tensor("hqT", [E, BS], F16, kind="ExternalInput")
    hkT = nc.dram_tensor("hkT", [E, BS], F16, kind="ExternalInput")
    w1 = nc.dram_tensor("w1", [E, E], F16, kind="ExternalInput")
    wk1 = nc.dram_tensor("wk1", [E, E], F16, kind="ExternalInput")
    w2 = nc.dram_tensor("w2", [E, D], F16, kind="ExternalInput")
    wk2 = nc.dram_tensor("wk2", [E, D], F16, kind="ExternalInput")
    wc = nc.dram_tensor("wc", [E, NCLS], F16, kind="ExternalInput")
    bp1 = nc.dram_tensor("bp1", [E], F32, kind="ExternalInput")
    bkp1 = nc.dram_tensor("bkp1", [E], F32, kind="ExternalInput")
    bp2 = nc.dram_tensor("bp2", [D], F32, kind="ExternalInput")
    bkp2 = nc.dram_tensor("bkp2", [D], F32, kind="ExternalInput")
    bcls = nc.dram_tensor("bcls", [NCLS], F32, kind="ExternalInput")
    qslT = nc.dram_tensor("qslT", [D, KS], F16, kind="ExternalInput")
    cslT = nc.dram_tensor("cslT", [D, KS], F16, kind="ExternalInput")
    cnat = nc.dram_tensor("cnat", [KS, D], BF16, kind="ExternalInput")
    labs = nc.dram_tensor("labs", [KS], F32, kind="ExternalInput")
    clab = nc.dram_tensor("clab", [BS], F32, kind="ExternalInput")

    lneg = nc.dram_tensor("lneg", [B, KS], F32, kind="ExternalOutput")
    stats = nc.dram_tensor("stats", [BS, 32], F32, kind="ExternalOutput")

    groups = [list(range(NCORES))]

    with tile.TileContext(nc) as tc, ExitStack() as ctx:
        consts = ctx.enter_context(tc.tile_pool(name="consts", bufs=1))
        persist = ctx.enter_context(tc.tile_pool(name="persist", bufs=1))
        dram = ctx.enter_context(tc.tile_pool(name="dram", bufs=1, space="DRAM"))

        # constants
        iota_i = consts.tile([128, NI], F32)
        nc.gpsimd.iota(iota_i, pattern=[[1, NI]], base=0, channel_multiplier=0,
                       allow_small_or_imprecise_dtypes=True)
        ones_col = consts.tile([128, 1], F32)
        nc.vector.memset(ones_col, 1.0)
        ones_row = consts.tile([1, 128], F32)
        nc.vector.memset(ones_row, 1.0)

        stats_sb = persist.tile([128, 32], F32)
        nc.vector.memset(stats_sb, 0.0)
        eps30 = consts.tile([1, 1], F32)
        nc.vector.memset(eps30, 1e-30)
        maebias = consts.tile([128, 1], F32)
        nc.vector.memset(maebias, 2.0 + 1e-6)

        # =========== Phase A: centroid partial sums over own K slice =========
        # cent_ps[d, i] += sum_k copy[k, d] * onehot[k, i]
        cent_sum = persist.tile([128, NI], F32)
        with tc.tile_pool(name="psA", bufs=1, space="PSUM") as psA, \
             tc.tile_pool(name="poolA", bufs=3) as poolA:
            cent_ps0 = psA.tile([128, 512], F32, tag="c0")
            cent_ps1 = psA.tile([128, 512], F32, tag="c1")
            nkt = KS // 128  # 64
            for kt in range(nkt):
                cpt = poolA.tile([128, D], BF16, tag="cpt")
                nc.sync.dma_start(out=cpt, in_=cnat[kt * 128:(kt + 1) * 128, :])
                labc = poolA.tile([128, 1], F32, tag="labc")
                nc.sync.dma_start(
                    out=labc,
                    in_=labs[kt * 128:(kt + 1) * 128].rearrange("(p o) -> p o", o=1))
                oh = poolA.tile([128, NI], BF16, tag="oh")
                nc.vector.tensor_scalar(oh, iota_i, labc, None, OP.is_equal)
                nc.tensor.matmul(cent_ps0, lhsT=cpt, rhs=oh[:, 0:512],
                                 start=(kt == 0), stop=(kt == nkt - 1))
                nc.tensor.matmul(cent_ps1, lhsT=cpt, rhs=oh[:, 512:NI],
                                 start=(kt == 0), stop=(kt == nkt - 1))
            nc.scalar.copy(cent_sum[:, 0:512], cent_ps0)
            nc.scalar.copy(cent_sum[:, 512:NI], cent_ps1)

        cent_in = dram.tile([128, NI], F32)
        cent_out = dram.tile([128, NI], F32)
        nc.sync.dma_start(out=cent_in, in_=cent_sum)
        nc.gpsimd.collective_compute(
            "AllReduce", OP.add, replica_groups=groups,
            ins=[cent_in.opt()], outs=[cent_out.opt()])
        centall = persist.tile([128, NI], F32)
        nc.sync.dma_start(out=centall, in_=cent_out)

        # =========== Phase B: projections (own 128 batch rows) ==============
        hq_sb = persist.tile([128, ET, BS], F16)
        nc.sync.dma_start(out=hq_sb, in_=hqT.rearrange("(t p) b -> p t b", p=128))
        hk_sb = persist.tile([128, ET, BS], F16)
        nc.sync.dma_start(out=hk_sb, in_=hkT.rearrange("(t p) b -> p t b", p=128))
        w2_sb = persist.tile([128, ET, D], F16)
        nc.sync.dma_start(out=w2_sb, in_=w2.rearrange("(t p) d -> p t d", p=128))
        wk2_sb = persist.tile([128, ET, D], F16)
        nc.sync.dma_start(out=wk2_sb, in_=wk2.rearrange("(t p) d -> p t d", p=128))
        bp1_sb = persist.tile([128, ET], F32)
        nc.sync.dma_start(out=bp1_sb, in_=bp1.rearrange("(t p) -> p t", p=128))
        bkp1_sb = persist.tile([128, ET], F32)
        nc.sync.dma_start(out=bkp1_sb, in_=bkp1.rearrange("(t p) -> p t", p=128))
        bp2_sb = persist.tile([128, 1], F32)
        nc.sync.dma_start(out=bp2_sb, in_=bp2.rearrange("(p o) -> p o", o=1))
        bkp2_sb = persist.tile([128, 1], F32)
        nc.sync.dma_start(out=bkp2_sb, in_=bkp2.rearrange("(p o) -> p o", o=1))

        qraw = persist.tile([128, BS], F32)   # [d, b] fp32, own shard
        kraw = persist.tile([128, BS], F32)

        with tc.tile_pool(name="psB", bufs=1, space="PSUM") as psB, \
             tc.tile_pool(name="poolB", bufs=3) as poolB, \
             tc.tile_pool(name="z1", bufs=1) as z1pool:
            ps_small = psB.tile([128, 16], F32, tag="small")
            for branch in range(2):
                wsrc = w1 if branch == 0 else wk1
                hsb = hq_sb if branch == 0 else hk_sb
                b1sb = bp1_sb if branch == 0 else bkp1_sb
                w2sb = w2_sb if branch == 0 else wk2_sb
                b2sb = bp2_sb if branch == 0 else bkp2_sb
                zout = qraw if branch == 0 else kraw
                z1_sb = z1pool.tile([128, ET, BS], F16, tag=f"z1_{branch}")
                for ot in range(ET):
                    wcol = poolB.tile([128, ET, 128], F16, tag="wcol")
                    nc.sync.dma_start(
                        out=wcol,
                        in_=wsrc[:, ot * 128:(ot + 1) * 128]
                        .rearrange("(t p) m -> p t m", p=128))
                    zps = psB.tile([128, BS], F32, tag="zps")
                    for et in range(ET):
                        nc.tensor.matmul(zps, lhsT=wcol[:, et, :],
                                         rhs=hsb[:, et, :],
                                         start=(et == 0), stop=(et == ET - 1))
                    nc.scalar.activation(z1_sb[:, ot, :], zps, AF.Relu,
                                         bias=b1sb[:, ot:ot + 1], scale=1.0)
                qps = psB.tile([128, BS], F32, tag="zps")
                for et in range(ET):
                    nc.tensor.matmul(qps, lhsT=w2sb[:, et, :], rhs=z1_sb[:, et, :],
                                     start=(et == 0), stop=(et == ET - 1))
                nc.scalar.activation(zout, qps, AF.Identity,
                                     bias=b2sb[:, 0:1], scale=1.0)

            # own-shard norms and l_pos
            sq = poolB.tile([128, BS], F32, tag="sqloc")
            nc.vector.tensor_mul(sq, qraw, qraw)
            nc.tensor.matmul(ps_small[:, 8:9], lhsT=sq, rhs=ones_col)
            sk = poolB.tile([128, BS], F32, tag="sqloc")
            nc.vector.tensor_mul(sk, kraw, kraw)
            nc.tensor.matmul(ps_small[:, 9:10], lhsT=sk, rhs=ones_col)
            pq = poolB.tile([128, BS], F32, tag="sqloc")
            nc.vector.tensor_mul(pq, qraw, kraw)
            nc.tensor.matmul(ps_small[:, 10:11], lhsT=pq, rhs=ones_col)
            nrm2 = persist.tile([128, 4], F32)
            nc.scalar.activation(nrm2[:, 0:2], ps_small[:, 8:10], AF.Sqrt,
                                 bias=0.0, scale=1.0)
            rloc = persist.tile([128, 2], F32)
            nc.vector.reciprocal(rloc, nrm2[:, 0:2])
            # l_pos = (q.k) * rnq * rnk / T  -> stats col 25
            lposv = persist.tile([128, 1], F32)
            nc.vector.tensor_scalar(lposv, ps_small[:, 10:11],
                                    rloc[:, 0:1], rloc[:, 1:2], OP.mult, OP.mult)
            nc.vector.tensor_scalar_mul(stats_sb[:, 25:26], lposv, 1.0 / T)

        # AllGather raw q across cores -> [d, B] on every core
        ag_in = dram.tile([128, BS], F32)
        ag_out = dram.tile([NCORES, 128, BS], F32)
        nc.sync.dma_start(out=ag_in, in_=qraw)
        nc.gpsimd.collective_compute(
            "AllGather", OP.bypass, replica_groups=groups,
            ins=[ag_in.opt()], outs=[ag_out.opt()])
        qall = persist.tile([128, B], F32)
        for c in range(NCORES):
            nc.sync.dma_start(out=qall[:, c * BS:(c + 1) * BS], in_=ag_out[c])

        # global norms (row layout) and pre-scaled fp16 copies of q
        q16q = persist.tile([128, B], F16)   # q * rnq / T      (logits)
        q16c = persist.tile([128, B], F16)   # q * (-2 rnq)     (mae)
        with tc.tile_pool(name="psC", bufs=2, space="PSUM") as psC, \
             tc.tile_pool(name="poolC", bufs=2) as poolC:
            sqall = poolC.tile([128, B], F32, tag="sqall")
            nc.vector.tensor_mul(sqall, qall, qall)
            nrow_ps = psC.tile([1, B], F32, tag="nrow")
            nc.tensor.matmul(nrow_ps[:, 0:512], lhsT=ones_col, rhs=sqall[:, 0:512])
            nc.tensor.matmul(nrow_ps[:, 512:B], lhsT=ones_col, rhs=sqall[:, 512:B])
            nrow = poolC.tile([1, B], F32, tag="nrowsb")
            nc.scalar.activation(nrow[:, 0:512], nrow_ps[:, 0:512], AF.Sqrt,
                                 bias=0.0, scale=1.0)
            nc.scalar.activation(nrow[:, 512:B], nrow_ps[:, 512:B], AF.Sqrt,
                                 bias=0.0, scale=1.0)
            rrow = poolC.tile([1, B], F32, tag="rrow")
            nc.vector.reciprocal(rrow, nrow)
            rq_row = poolC.tile([1, B], F32, tag="rqrow")
            nc.vector.tensor_scalar_mul(rq_row, rrow, 1.0 / T)
            rc_row = poolC.tile([1, B], F32, tag="rcrow")
            nc.vector.tensor_scalar_mul(rc_row, rrow, -2.0)
            for half in range(2):
                sl = slice(half * 512, (half + 1) * 512)
                bq_ps = psC.tile([128, 512], F32, tag="bq")
                nc.tensor.matmul(bq_ps, lhsT=ones_row, rhs=rq_row[:, sl])
                nc.vector.tensor_mul(q16q[:, sl], qall[:, sl], bq_ps)
                bc_ps = psC.tile([128, 512], F32, tag="bq")
                nc.tensor.matmul(bc_ps, lhsT=ones_row, rhs=rc_row[:, sl])
                nc.vector.tensor_mul(q16c[:, sl], qall[:, sl], bc_ps)

            # ========= Phase C: centroid normalize + pseudo labels ==========
            sqc = poolC.tile([128, NI], F32, tag="sqall")
            nc.vector.tensor_mul(sqc, centall, centall)
            cn_ps = psC.tile([1, NI], F32, tag="nrow")
            nc.tensor.matmul(cn_ps[:, 0:512], lhsT=ones_col, rhs=sqc[:, 0:512])
            nc.tensor.matmul(cn_ps[:, 512:NI], lhsT=ones_col, rhs=sqc[:, 512:NI])
            cnn = poolC.tile([1, NI], F32, tag="nrowsb")
            nc.scalar.activation(cnn[:, 0:512], cn_ps[:, 0:512], AF.Sqrt,
                                 bias=eps30[0:1, 0:1], scale=1.0)
            nc.scalar.activation(cnn[:, 512:NI], cn_ps[:, 512:NI], AF.Sqrt,
                                 bias=eps30[0:1, 0:1], scale=1.0)
            crn = poolC.tile([1, NI], F32, tag="rrow")
            nc.vector.reciprocal(crn, cnn)
            centn = persist.tile([128, NI], F16)
            for half in range(2):
                sl = slice(half * 512, (half + 1) * 512)
                cb_ps = psC.tile([128, 512], F32, tag="bq")
                nc.tensor.matmul(cb_ps, lhsT=ones_row, rhs=crn[:, sl])
                nc.vector.tensor_mul(centn[:, sl], centall[:, sl], cb_ps)

            pseudo_i = persist.tile([128, NBT], F32)
            for bt in range(NBT):
                plog = poolC.tile([128, NI], F32, tag="plog")
                for half in range(2):
                    sl = slice(half * 512, (half + 1) * 512)
                    pl_ps = psC.tile([128, 512], F32, tag="bq")
                    nc.tensor.matmul(pl_ps, lhsT=q16q[:, bt * 128:(bt + 1) * 128],
                                     rhs=centn[:, sl])
                    nc.scalar.copy(plog[:, sl], pl_ps)
                mx8 = poolC.tile([128, 8], F32, tag="mx8")
                ix8 = poolC.tile([128, 8], mybir.dt.uint32, tag="ix8")
                nc.vector.max_with_indices(mx8, ix8, plog)
                nc.vector.tensor_copy(pseudo_i[:, bt:bt + 1], ix8[:, 0:1])
            # export pseudo (as f32) for host-side exact mask counts
            nc.vector.tensor_copy(stats_sb[:, 16:24], pseudo_i)

        # =========== Phase E: classification head (own rows) ============
        wc_sb = persist.tile([128, ET, NCLS], F16)
        nc.sync.dma_start(out=wc_sb, in_=wc.rearrange("(t p) c -> p t c", p=128))
        bcls_bc = persist.tile([128, NCLS], F32)
        nc.gpsimd.dma_start(
            out=bcls_bc,
            in_=bass.AP(tensor=bcls.ap().tensor, offset=0,
                        ap=[[0, 128]] + list(bcls.ap().ap)))
        clab_sb = persist.tile([128, 1], F32)
        nc.sync.dma_start(out=clab_sb, in_=clab.rearrange("(p o) -> p o", o=1))
        with tc.tile_pool(name="psE", bufs=1, space="PSUM") as psE, \
             tc.tile_pool(name="poolE", bufs=1) as poolE:
            cls_ps = psE.tile([128, NCLS], F32, tag="cls")
            for et in range(ET):
                nc.tensor.matmul(cls_ps, lhsT=hq_sb[:, et, :], rhs=wc_sb[:, et, :],
                                 start=(et == 0), stop=(et == ET - 1))
            coarse = poolE.tile([128, NCLS], F32, tag="coarse")
            nc.vector.tensor_add(coarse, cls_ps, bcls_bc)
            mxc = poolE.tile([128, 1], F32, tag="mxc")
            nc.vector.tensor_reduce(mxc, coarse, axis=AX.X, op=OP.max)
            mxn = poolE.tile([128, 1], F32, tag="mxn")
            nc.vector.tensor_scalar_mul(mxn, mxc, -1.0)
            es = poolE.tile([128, NCLS], BF16, tag="es")
            sume = poolE.tile([128, 1], F32, tag="sume")
            nc.scalar.activation(es, coarse, AF.Exp, bias=mxn[:, 0:1], scale=1.0,
                                 accum_out=sume)
            lse = poolE.tile([128, 1], F32, tag="lse")
            nc.scalar.activation(lse, sume, AF.Ln, bias=0.0, scale=1.0)
            ohc = poolE.tile([128, NCLS], F32, tag="ohc")
            nc.vector.tensor_scalar(ohc, iota_i[:, 0:NCLS], clab_sb, None,
                                    OP.is_equal)
            scrc = poolE.tile([128, NCLS], F32, tag="ohscr")
            picked = poolE.tile([128, 1], F32, tag="picked")
            nc.vector.tensor_mul(scrc, coarse, ohc)
            nc.vector.tensor_reduce(picked, scrc, axis=AX.X, op=OP.add)
            t1 = poolE.tile([128, 1], F32, tag="t1")
            nc.vector.tensor_sub(t1, picked, mxc)
            nc.vector.tensor_sub(stats_sb[:, 24:25], t1, lse)

        # =========== Phase D: the B x KS similarity / mae / mask loop =======
        acc_ms = persist.tile([128, NBT * NCH], F32)   # masked mae sums
        acc_mt = persist.tile([128, NBT * NCH], F32)   # total mae sums
        with tc.tile_pool(name="psD", bufs=2, space="PSUM") as psD, \
             tc.tile_pool(name="poolD", bufs=3) as poolD, \
             tc.tile_pool(name="outD", bufs=4) as outD:
            for ch in range(NCH):
                c0 = ch * CH
                qt = poolD.tile([128, CH], F16, tag="qt")
                nc.sync.dma_start(out=qt, in_=qslT[:, c0:c0 + CH])
                ct = poolD.tile([128, CH], F16, tag="ct")
                nc.sync.dma_start(out=ct, in_=cslT[:, c0:c0 + CH])
                labb = poolD.tile([128, CH], F32, tag="labb")
                lab_sl = labs[c0:c0 + CH]
                nc.gpsimd.dma_start(
                    out=labb,
                    in_=bass.AP(tensor=lab_sl.tensor, offset=lab_sl.offset,
                                ap=[[0, 128]] + list(lab_sl.ap)))
                for bt in range(NBT):
                    bsl = slice(bt * 128, (bt + 1) * 128)
                    ci = bt * NCH + ch
                    sq_ps = psD.tile([128, CH], F32, tag="sq")
                    nc.tensor.matmul(sq_ps[:, 0:512], lhsT=q16q[:, bsl],
                                     rhs=qt[:, 0:512])
                    nc.tensor.matmul(sq_ps[:, 512:CH], lhsT=q16q[:, bsl],
                                     rhs=qt[:, 512:CH])
                    lgch = outD.tile([128, CH], F32, tag="lgch")
                    nc.scalar.copy(lgch, sq_ps)
                    nc.scalar.dma_start(out=lneg[bsl, c0:c0 + CH], in_=lgch)
                    sc_ps = psD.tile([128, CH], F32, tag="sc")
                    nc.tensor.matmul(sc_ps[:, 0:512], lhsT=q16c[:, bsl],
                                     rhs=ct[:, 0:512])
                    nc.tensor.matmul(sc_ps[:, 512:CH], lhsT=q16c[:, bsl],
                                     rhs=ct[:, 512:CH])
                    mae = poolD.tile([128, CH], BF16, tag="mae")
                    nc.scalar.activation(mae, sc_ps, AF.Sqrt,
                                         bias=maebias[:, 0:1], scale=1.0,
                                         accum_out=acc_mt[:, ci:ci + 1])
                    scr = poolD.tile([128, CH], BF16, tag="scr")
                    nc.vector.scalar_tensor_tensor(
                        out=scr, in0=labb, scalar=pseudo_i[:, bt:bt + 1],
                        in1=mae, op0=OP.is_equal, op1=OP.mult,
                        accum_out=acc_ms[:, ci:ci + 1])

            # =========== Final: fold chunk accumulators, write stats ========
            for bt in range(NBT):
                sl = slice(bt * NCH, (bt + 1) * NCH)
                nc.vector.tensor_reduce(stats_sb[:, bt:bt + 1], acc_ms[:, sl],
                                        axis=AX.X, op=OP.add)
                nc.vector.tensor_reduce(stats_sb[:, 8 + bt:9 + bt], acc_mt[:, sl],
                                        axis=AX.X, op=OP.add)
            nc.scalar.dma_start(out=stats.ap(), in_=stats_sb)

    nc.compile()
    return nc


_NC_CACHE = None


def _get_nc():
    global _NC_CACHE
    if _NC_CACHE is None:
        _NC_CACHE = build_kernel()
    return _NC_CACHE


def kernel(h_q, h_k, W_cls, b_cls, W_p1, b_p1, W_p2, b_p2,
           Wk_p1, bk_p1, Wk_p2, bk_p2, queue_emb, queue_emb_copy,
           info_label, coarse_labs):
    nc = _get_nc()
    import ml_dtypes
    bf16_t = ml_dtypes.bfloat16
    f16 = np.float16
    f32 = np.float32
    ca = np.ascontiguousarray

    h_q = np.asarray(h_q, f32)
    h_k = np.asarray(h_k, f32)
    queue_emb = np.asarray(queue_emb, f32)
    queue_emb_copy = np.asarray(queue_emb_copy, f32)
    info_label = np.asarray(info_label).astype(np.int32)
    coarse_labs = np.asarray(coarse_labs).astype(np.int32)

    w1 = ca(np.asarray(W_p1, f16))
    wk1 = ca(np.asarray(Wk_p1, f16))
    w2 = ca(np.asarray(W_p2, f16))
    wk2 = ca(np.asarray(Wk_p2, f16))
    wc = ca(np.asarray(W_cls, f16))
    bp1 = ca(np.asarray(b_p1, f32))
    bkp1 = ca(np.asarray(bk_p1, f32))
    bp2 = ca(np.asarray(b_p2, f32))
    bkp2 = ca(np.asarray(bk_p2, f32))
    bcv = ca(np.asarray(b_cls, f32))
    queueT = ca(queue_emb.T.astype(f16))        # [D, K]
    copyT = ca(queue_emb_copy.T.astype(f16))    # [D, K]

    in_maps = []
    for c in range(NCORES):
        bs = slice(c * BS, (c + 1) * BS)
        ks = slice(c * KS, (c + 1) * KS)
        in_maps.append({
            "hqT": ca(h_q[bs].T.astype(f16)),
            "hkT": ca(h_k[bs].T.astype(f16)),
            "w1": w1, "wk1": wk1, "w2": w2, "wk2": wk2, "wc": wc,
            "bp1": bp1, "bkp1": bkp1, "bp2": bp2, "bkp2": bkp2, "bcls": bcv,
            "qslT": ca(queueT[:, ks]),
            "cslT": ca(copyT[:, ks]),
            "cnat": ca(queue_emb_copy[ks].astype(bf16_t)),
            "labs": ca(info_label[ks].astype(f32)),
            "clab": ca(coarse_labs[bs].astype(f32)),
        })

    res = run_bass_kernel_spmd(nc, in_maps, list(range(NCORES)))
    results = res.results

    # ---- host-side gather / final scalar math ----
    logits = np.empty((B, 1 + K), f32)
    for c in range(NCORES):
        logits[:, 1 + c * KS: 1 + (c + 1) * KS] = results[c]["lneg"]
    stats = [results[c]["stats"] for c in range(NCORES)]
    for c in range(NCORES):
        logits[c * BS:(c + 1) * BS, 0] = stats[c][:, 25]

    # masked mae sums / total mae sums: partial over each core's K slice
    msum = np.zeros((B,), np.float64)
    mtot = np.zeros((B,), np.float64)
    for c in range(NCORES):
        msum += stats[c][:, 0:8].T.reshape(B).astype(np.float64)
        mtot += stats[c][:, 8:16].T.reshape(B).astype(np.float64)
    # pseudo labels (identical on every core; take core 0)
    pseudo = stats[0][:, 16:24].T.reshape(B).astype(np.int64)
    class_counts = np.bincount(info_label, minlength=NI)
    cnt = class_counts[pseudo].astype(np.float64)

    eps = 1e-6
    min_e = np.mean(msum / (cnt + eps))
    avg_inter = np.mean((mtot - msum) / (K - cnt + eps))
    dino_loss = np.float32(min_e + (2.0 - avg_inter))

    logp = np.concatenate([stats[c][:, 24] for c in range(NCORES)])
    cls_loss = np.float32(-logp.mean())

    labels = np.zeros((B,), np.int32)
    return logits, labels, dino_loss, cls_loss


# revision 11
# speedup vs baseline: 3.2165x; 1.1546x over previous
"""MoCo forward kernel for 8 Trainium2 NeuronCores.

Sharding:
  - Projections (h->q, h->k), l_pos, cls head: data-parallel over batch
    (128 rows per core), followed by an AllGather of raw q vectors.
  - Per-info-class centroid sums: sharded over the queue dim K
    (8192 rows per core) via a one-hot matmul, combined with an AllReduce.
  - The big B x K similarity / mask / MAE phase: sharded over K columns
    (each core computes all 1024 rows against its 8192 queue columns),
    which avoids replicating the 2x 32MB queue reads on every core.

All matmuls run in fp16 (same PE rate as bf16, 4x the mantissa) with fp32
PSUM accumulation; l2 normalization factors are folded into pre-scaled
copies of q so the inner loop is pure matmul + one ACT sqrt + one DVE
copy + one DVE masked-reduce + one GPSIMD compare per tile.
"""

import sys

sys.path.insert(0, '/opt/trn_rl_repo')

from contextlib import ExitStack

import numpy as np

import concourse.bass as bass
import concourse.mybir as mybir
import concourse.tile as tile
from concourse import bacc
from concourse.bass_utils import run_bass_kernel_spmd
from concourse.masks import make_identity

NCORES = 8
B, E, D, K = 1024, 2048, 128, 65536
NI, NCLS = 1024, 100
T = 0.07
BS = B // NCORES          # 128 batch rows per core
KS = K // NCORES          # 8192 queue rows/cols per core
CH = 1024                 # chunk width in the K loop
NCH = KS // CH            # 16 chunks per core
NBT = B // 128            # 8 batch tiles of 128
ET = E // 128             # 16 contraction tiles over EMBED

F16 = mybir.dt.float16
BF16 = mybir.dt.bfloat16
F32 = mybir.dt.float32
I32 = mybir.dt.int32
AX = mybir.AxisListType
OP = mybir.AluOpType
AF = mybir.ActivationFunctionType


def build_kernel():
    nc = bacc.Bacc('TRN2', target_bir_lowering=False, debug=False,
                   num_devices=NCORES)

    # ---- DRAM I/O ----
    hqT = nc.dram_tensor("hqT", [E, BS], F16, kind="ExternalInput")
    hkT = nc.dram_tensor("hkT", [E, BS], F16, kind="ExternalInput")
    w1 = nc.dram_tensor("w1", [E, E], F16, kind="ExternalInput")
    wk1 = nc.dram_tensor("wk1", [E, E], F16, kind="ExternalInput")
    w2 = nc.dram_tensor("w2", [E, D], F16, kind="ExternalInput")
    wk2 = nc.dram_tensor("wk2", [E, D], F16, kind="ExternalInput")
    wc = nc.dram_tensor("wc", [E, NCLS], F16, kind="ExternalInput")
    bp1 = nc.dram_tensor("bp1", [E], F16, kind="ExternalInput")
    bkp1 = nc.dram_tensor("bkp1", [E], F16, kind="ExternalInput")
    bp2 = nc.dram_tensor("bp2", [D], F32, kind="ExternalInput")
    bkp2 = nc.dram_tensor("bkp2", [D], F32, kind="ExternalInput")
    bcls = nc.dram_tensor("bcls", [NCLS], F32, kind="ExternalInput")
    qslT = nc.dram_tensor("qslT", [D, KS], F16, kind="ExternalInput")
    cslT = nc.dram_tensor("cslT", [D, KS], F16, kind="ExternalInput")
    cnat = nc.dram_tensor("cnat", [KS, D], BF16, kind="ExternalInput")
    labs = nc.dram_tensor("labs", [KS], F32, kind="ExternalInput")
    clab = nc.dram_tensor("clab", [BS], F32, kind="ExternalInput")

    lneg = nc.dram_tensor("lneg", [B, KS], F32, kind="ExternalOutput")
    stats = nc.dram_tensor("stats", [BS, 32], F32, kind="ExternalOutput")

    groups = [list(range(NCORES))]

    with tile.TileContext(nc) as tc, ExitStack() as ctx:
        consts = ctx.enter_context(tc.tile_pool(name="consts", bufs=1))
        persist = ctx.enter_context(tc.tile_pool(name="persist", bufs=1))
        dram = ctx.enter_context(tc.tile_pool(name="dram", bufs=1, space="DRAM"))

        # constants
        iota_i = consts.tile([128, NI], F32)
        nc.gpsimd.iota(iota_i, pattern=[[1, NI]], base=0, channel_multiplier=0,
                       allow_small_or_imprecise_dtypes=True)
        ones_col = consts.tile([128, 1], F32)
        nc.vector.memset(ones_col, 1.0)
        ones_row = consts.tile([1, 128], F32)
        nc.vector.memset(ones_row, 1.0)

        stats_sb = persist.tile([128, 32], F32)
        nc.vector.memset(stats_sb, 0.0)
        eps30 = consts.tile([1, 1], F32)
        nc.vector.memset(eps30, 1e-30)
        maebias = consts.tile([128, 1], F32)
        nc.vector.memset(maebias, 2.0 + 1e-6)

        # =========== Phase A: centroid partial sums over own K slice =========
        # cent_ps[d, i] += sum_k copy[k, d] * onehot[k, i]
        cent_sum = persist.tile([128, NI], F32)
        with tc.tile_pool(name="psA", bufs=1, space="PSUM") as psA, \
             tc.tile_pool(name="poolA", bufs=3) as poolA:
            cent_ps0 = psA.tile([128, 512], F32, tag="c0")
            cent_ps1 = psA.tile([128, 512], F32, tag="c1")
            nkt = KS // 128  # 64
            for kt in range(nkt):
                cpt = poolA.tile([128, D], BF16, tag="cpt")
                nc.sync.dma_start(out=cpt, in_=cnat[kt * 128:(kt + 1) * 128, :])
                labc = poolA.tile([128, 1], F32, tag="labc")
                nc.sync.dma_start(
                    out=labc,
                    in_=labs[kt * 128:(kt + 1) * 128].rearrange("(p o) -> p o", o=1))
                oh = poolA.tile([128, NI], BF16, tag="oh")
                nc.vector.tensor_scalar(oh, iota_i, labc, None, OP.is_equal)
                nc.tensor.matmul(cent_ps0, lhsT=cpt, rhs=oh[:, 0:512],
                                 start=(kt == 0), stop=(kt == nkt - 1))
                nc.tensor.matmul(cent_ps1, lhsT=cpt, rhs=oh[:, 512:NI],
                                 start=(kt == 0), stop=(kt == nkt - 1))
            nc.scalar.copy(cent_sum[:, 0:512], cent_ps0)
            nc.scalar.copy(cent_sum[:, 512:NI], cent_ps1)

        cent_in = dram.tile([128, NI], F32)
        cent_out = dram.tile([128, NI], F32)
        nc.sync.dma_start(out=cent_in, in_=cent_sum)
        nc.gpsimd.collective_compute(
            "AllReduce", OP.add, replica_groups=groups,
            ins=[cent_in.opt()], outs=[cent_out.opt()])
        centall = persist.tile([128, NI], F32)
        nc.sync.dma_start(out=centall, in_=cent_out)

        # =========== Phase B: projections (own 128 batch rows) ==============
        # Layer 1 runs h-stationary (16 LDWs instead of 256): for each e-tile,
        # lhsT = hT tile, moving = a 2048-wide row block of W1. The bias is
        # seeded into PSUM with a ones-row matmul, giving z1 = relu(h@W1+b1)
        # in [b, oe] layout, which PE-transposes to [oe, b] for layer 2.
        hq_sb = persist.tile([128, ET, BS], F16)
        nc.sync.dma_start(out=hq_sb, in_=hqT.rearrange("(t p) b -> p t b", p=128))
        hk_sb = persist.tile([128, ET, BS], F16)
        nc.sync.dma_start(out=hk_sb, in_=hkT.rearrange("(t p) b -> p t b", p=128))
        w2_sb = persist.tile([128, ET, D], F16)
        nc.sync.dma_start(out=w2_sb, in_=w2.rearrange("(t p) d -> p t d", p=128))
        wk2_sb = persist.tile([128, ET, D], F16)
        nc.sync.dma_start(out=wk2_sb, in_=wk2.rearrange("(t p) d -> p t d", p=128))
        b1row_q = persist.tile([1, E], F16)
        nc.sync.dma_start(out=b1row_q, in_=bp1.rearrange("(o e) -> o e", o=1))
        b1row_k = persist.tile([1, E], F16)
        nc.sync.dma_start(out=b1row_k, in_=bkp1.rearrange("(o e) -> o e", o=1))
        bp2_sb = persist.tile([128, 1], F32)
        nc.sync.dma_start(out=bp2_sb, in_=bp2.rearrange("(p o) -> p o", o=1))
        bkp2_sb = persist.tile([128, 1], F32)
        nc.sync.dma_start(out=bkp2_sb, in_=bkp2.rearrange("(p o) -> p o", o=1))
        ones1 = consts.tile([1, 128], F16)
        nc.vector.memset(ones1, 1.0)
        ident = consts.tile([128, 128], F16)
        make_identity(nc, ident)

        qraw = persist.tile([128, BS], F32)   # [d, b] fp32, own shard
        kraw = persist.tile([128, BS], F32)
        ag_in = dram.tile([128, BS], F32)
        ag_out = dram.tile([NCORES, 128, BS], F32)

        with tc.tile_pool(name="psB", bufs=1, space="PSUM") as psB, \
             tc.tile_pool(name="psT", bufs=2, space="PSUM") as psT, \
             tc.tile_pool(name="poolB", bufs=3) as poolB, \
             tc.tile_pool(name="z1", bufs=1) as z1pool:
            ps_small = psB.tile([128, 16], F32, tag="small")
            zps = [psB.tile([128, 512], F32, tag=f"zc{c}", name=f"zps{c}") for c in range(4)]
            for branch in range(2):
                wsrc = w1 if branch == 0 else wk1
                hsb = hq_sb if branch == 0 else hk_sb
                b1r = b1row_q if branch == 0 else b1row_k
                w2sb = w2_sb if branch == 0 else wk2_sb
                b2sb = bp2_sb if branch == 0 else bkp2_sb
                zout = qraw if branch == 0 else kraw
                for c in range(4):
                    nc.tensor.matmul(zps[c], lhsT=ones1,
                                     rhs=b1r[:, c * 512:(c + 1) * 512],
                                     start=True, stop=False)
                for et in range(ET):
                    wrow = poolB.tile([128, E], F16, tag="wrow")
                    nc.sync.dma_start(out=wrow,
                                      in_=wsrc[et * 128:(et + 1) * 128, :])
                    for c in range(4):
                        nc.tensor.matmul(zps[c], lhsT=hsb[:, et, :],
                                         rhs=wrow[:, c * 512:(c + 1) * 512],
                                         start=False, stop=(et == ET - 1))
                z1f = z1pool.tile([128, E], F16, tag=f"z1f_{branch}")
                for c in range(4):
                    nc.scalar.activation(z1f[:, c * 512:(c + 1) * 512], zps[c],
                                         AF.Relu, bias=0.0, scale=1.0)
                z1t = z1pool.tile([128, ET, BS], F16, tag=f"z1t_{branch}")
                for ot in range(ET):
                    tp = psT.tile([128, 128], F16, tag="tp")
                    nc.tensor.transpose(tp, z1f[:, ot * 128:(ot + 1) * 128], ident)
                    nc.scalar.copy(z1t[:, ot, :], tp)
                qps = psB.tile([128, BS], F32, tag="qps")
                for et in range(ET):
                    nc.tensor.matmul(qps, lhsT=w2sb[:, et, :], rhs=z1t[:, et, :],
                                     start=(et == 0), stop=(et == ET - 1))
                nc.scalar.activation(zout, qps, AF.Identity,
                                     bias=b2sb[:, 0:1], scale=1.0)
                if branch == 0:
                    # AllGather of raw q dispatched while the k branch computes
                    nc.sync.dma_start(out=ag_in, in_=qraw)
                    nc.gpsimd.collective_compute(
                        "AllGather", OP.bypass, replica_groups=groups,
                        ins=[ag_in.opt()], outs=[ag_out.opt()])

            # own-shard norms and l_pos
            sq = poolB.tile([128, BS], F32, tag="sqloc")
            nc.vector.tensor_mul(sq, qraw, qraw)
            nc.tensor.matmul(ps_small[:, 8:9], lhsT=sq, rhs=ones_col)
            sk = poolB.tile([128, BS], F32, tag="sqloc")
            nc.vector.tensor_mul(sk, kraw, kraw)
            nc.tensor.matmul(ps_small[:, 9:10], lhsT=sk, rhs=ones_col)
            pq = poolB.tile([128, BS], F32, tag="sqloc")
            nc.vector.tensor_mul(pq, qraw, kraw)
            nc.tensor.matmul(ps_small[:, 10:11], lhsT=pq, rhs=ones_col)
            nrm2 = persist.tile([128, 4], F32)
            nc.scalar.activation(nrm2[:, 0:2], ps_small[:, 8:10], AF.Sqrt,
                                 bias=0.0, scale=1.0)
            rloc = persist.tile([128, 2], F32)
            nc.vector.reciprocal(rloc, nrm2[:, 0:2])
            # l_pos = (q.k) * rnq * rnk / T  -> stats col 25
            lposv = persist.tile([128, 1], F32)
            nc.vector.tensor_scalar(lposv, ps_small[:, 10:11],
                                    rloc[:, 0:1], rloc[:, 1:2], OP.mult, OP.mult)
            nc.vector.tensor_scalar_mul(stats_sb[:, 25:26], lposv, 1.0 / T)

        qall = persist.tile([128, B], F32)
        for c in range(NCORES):
            nc.sync.dma_start(out=qall[:, c * BS:(c + 1) * BS], in_=ag_out[c])

        # global norms (row layout) and pre-scaled fp16 copies of q
        q16q = persist.tile([128, B], F16)   # q * rnq / T      (logits)
        q16c = persist.tile([128, B], F16)   # q * (-2 rnq)     (mae)
        with tc.tile_pool(name="psC", bufs=2, space="PSUM") as psC, \
             tc.tile_pool(name="poolC", bufs=2) as poolC:
            sqall = poolC.tile([128, B], F32, tag="sqall")
            nc.vector.tensor_mul(sqall, qall, qall)
            nrow_ps = psC.tile([1, B], F32, tag="nrow")
            nc.tensor.matmul(nrow_ps[:, 0:512], lhsT=ones_col, rhs=sqall[:, 0:512])
            nc.tensor.matmul(nrow_ps[:, 512:B], lhsT=ones_col, rhs=sqall[:, 512:B])
            nrow = poolC.tile([1, B], F32, tag="nrowsb")
            nc.scalar.activation(nrow[:, 0:512], nrow_ps[:, 0:512], AF.Sqrt,
                                 bias=0.0, scale=1.0)
            nc.scalar.activation(nrow[:, 512:B], nrow_ps[:, 512:B], AF.Sqrt,
                                 bias=0.0, scale=1.0)
            rrow = poolC.tile([1, B], F32, tag="rrow")
            nc.vector.reciprocal(rrow, nrow)
            rq_row = poolC.tile([1, B], F32, tag="rqrow")
            nc.vector.tensor_scalar_mul(rq_row, rrow, 1.0 / T)
            rc_row = poolC.tile([1, B], F32, tag="rcrow")
            nc.vector.tensor_scalar_mul(rc_row, rrow, -2.0)
            for half in range(2):
                sl = slice(half * 512, (half + 1) * 512)
                bq_ps = psC.tile([128, 512], F32, tag="bq")
                nc.tensor.matmul(bq_ps, lhsT=ones_row, rhs=rq_row[:, sl])
                nc.vector.tensor_mul(q16q[:, sl], qall[:, sl], bq_ps)
                bc_ps = psC.tile([128, 512], F32, tag="bq")
                nc.tensor.matmul(bc_ps, lhsT=ones_row, rhs=rc_row[:, sl])
                nc.vector.tensor_mul(q16c[:, sl], qall[:, sl], bc_ps)

            # ========= Phase C: centroid normalize + pseudo labels ==========
            sqc = poolC.tile([128, NI], F32, tag="sqall")
            nc.vector.tensor_mul(sqc, centall, centall)
            cn_ps = psC.tile([1, NI], F32, tag="nrow")
            nc.tensor.matmul(cn_ps[:, 0:512], lhsT=ones_col, rhs=sqc[:, 0:512])
            nc.tensor.matmul(cn_ps[:, 512:NI], lhsT=ones_col, rhs=sqc[:, 512:NI])
            cnn = poolC.tile([1, NI], F32, tag="nrowsb")
            nc.scalar.activation(cnn[:, 0:512], cn_ps[:, 0:512], AF.Sqrt,
                                 bias=eps30[0:1, 0:1], scale=1.0)
            nc.scalar.activation(cnn[:, 512:NI], cn_ps[:, 512:NI], AF.Sqrt,
                                 bias=eps30[0:1, 0:1], scale=1.0)
            crn = poolC.tile([1, NI], F32, tag="rrow")
            nc.vector.reciprocal(crn, cnn)
            centn = persist.tile([128, NI], F16)
            for half in range(2):
                sl = slice(half * 512, (half + 1) * 512)
                cb_ps = psC.tile([128, 512], F32, tag="bq")
                nc.tensor.matmul(cb_ps, lhsT=ones_row, rhs=crn[:, sl])
                nc.vector.tensor_mul(centn[:, sl], centall[:, sl], cb_ps)

            pseudo_i = persist.tile([128, NBT], F32)
            for bt in range(NBT):
                plog = poolC.tile([128, NI], F32, tag="plog")
                for half in range(2):
                    sl = slice(half * 512, (half + 1) * 512)
                    pl_ps = psC.tile([128, 512], F32, tag="bq")
                    nc.tensor.matmul(pl_ps, lhsT=q16q[:, bt * 128:(bt + 1) * 128],
                                     rhs=centn[:, sl])
                    nc.scalar.copy(plog[:, sl], pl_ps)
                mx8 = poolC.tile([128, 8], F32, tag="mx8")
                ix8 = poolC.tile([128, 8], mybir.dt.uint32, tag="ix8")
                nc.vector.max_with_indices(mx8, ix8, plog)
                nc.vector.tensor_copy(pseudo_i[:, bt:bt + 1], ix8[:, 0:1])
            # export pseudo (as f32) for host-side exact mask counts
            nc.vector.tensor_copy(stats_sb[:, 16:24], pseudo_i)

        # =========== Phase E: classification head (own rows) ============
        wc_sb = persist.tile([128, ET, NCLS], F16)
        nc.sync.dma_start(out=wc_sb, in_=wc.rearrange("(t p) c -> p t c", p=128))
        bcls_bc = persist.tile([128, NCLS], F32)
        nc.gpsimd.dma_start(
            out=bcls_bc,
            in_=bass.AP(tensor=bcls.ap().tensor, offset=0,
                        ap=[[0, 128]] + list(bcls.ap().ap)))
        clab_sb = persist.tile([128, 1], F32)
        nc.sync.dma_start(out=clab_sb, in_=clab.rearrange("(p o) -> p o", o=1))
        with tc.tile_pool(name="psE", bufs=1, space="PSUM") as psE, \
             tc.tile_pool(name="poolE", bufs=1) as poolE:
            cls_ps = psE.tile([128, NCLS], F32, tag="cls")
            for et in range(ET):
                nc.tensor.matmul(cls_ps, lhsT=hq_sb[:, et, :], rhs=wc_sb[:, et, :],
                                 start=(et == 0), stop=(et == ET - 1))
            coarse = poolE.tile([128, NCLS], F32, tag="coarse")
            nc.vector.tensor_add(coarse, cls_ps, bcls_bc)
            mxc = poolE.tile([128, 1], F32, tag="mxc")
            nc.vector.tensor_reduce(mxc, coarse, axis=AX.X, op=OP.max)
            mxn = poolE.tile([128, 1], F32, tag="mxn")
            nc.vector.tensor_scalar_mul(mxn, mxc, -1.0)
            es = poolE.tile([128, NCLS], BF16, tag="es")
            sume = poolE.tile([128, 1], F32, tag="sume")
            nc.scalar.activation(es, coarse, AF.Exp, bias=mxn[:, 0:1], scale=1.0,
                                 accum_out=sume)
            lse = poolE.tile([128, 1], F32, tag="lse")
            nc.scalar.activation(lse, sume, AF.Ln, bias=0.0, scale=1.0)
            ohc = poolE.tile([128, NCLS], F32, tag="ohc")
            nc.vector.tensor_scalar(ohc, iota_i[:, 0:NCLS], clab_sb, None,
                                    OP.is_equal)
            scrc = poolE.tile([128, NCLS], F32, tag="ohscr")
            picked = poolE.tile([128, 1], F32, tag="picked")
            nc.vector.tensor_mul(scrc, coarse, ohc)
            nc.vector.tensor_reduce(picked, scrc, axis=AX.X, op=OP.add)
            t1 = poolE.tile([128, 1], F32, tag="t1")
            nc.vector.tensor_sub(t1, picked, mxc)
            nc.vector.tensor_sub(stats_sb[:, 24:25], t1, lse)

        # =========== Phase D: the B x KS similarity / mae / mask loop =======
        acc_ms = persist.tile([128, NBT * NCH], F32)   # masked mae sums
        acc_mt = persist.tile([128, NBT * NCH], F32)   # total mae sums
        with tc.tile_pool(name="psD", bufs=2, space="PSUM") as psD, \
             tc.tile_pool(name="poolD", bufs=3) as poolD, \
             tc.tile_pool(name="outD", bufs=4) as outD:
            for ch in range(NCH):
                c0 = ch * CH
                qt = poolD.tile([128, CH], F16, tag="qt")
                nc.sync.dma_start(out=qt, in_=qslT[:, c0:c0 + CH])
                ct = poolD.tile([128, CH], F16, tag="ct")
                nc.sync.dma_start(out=ct, in_=cslT[:, c0:c0 + CH])
                labb = poolD.tile([128, CH], F32, tag="labb")
                lab_sl = labs[c0:c0 + CH]
                nc.gpsimd.dma_start(
                    out=labb,
                    in_=bass.AP(tensor=lab_sl.tensor, offset=lab_sl.offset,
                                ap=[[0, 128]] + list(lab_sl.ap)))
                for bt in range(NBT):
                    bsl = slice(bt * 128, (bt + 1) * 128)
                    ci = bt * NCH + ch
                    sq_ps = psD.tile([128, CH], F32, tag="sq")
                    nc.tensor.matmul(sq_ps[:, 0:512], lhsT=q16q[:, bsl],
                                     rhs=qt[:, 0:512])
                    nc.tensor.matmul(sq_ps[:, 512:CH], lhsT=q16q[:, bsl],
                                     rhs=qt[:, 512:CH])
                    lgch = outD.tile([128, CH], F32, tag="lgch")
                    if (ch * NBT + bt) % 2 == 0:
                        nc.vector.tensor_copy(lgch, sq_ps)
                    else:
                        nc.scalar.copy(lgch, sq_ps)
                    nc.scalar.dma_start(out=lneg[bsl, c0:c0 + CH], in_=lgch)
                    sc_ps = psD.tile([128, CH], F32, tag="sc")
                    nc.tensor.matmul(sc_ps[:, 0:512], lhsT=q16c[:, bsl],
                                     rhs=ct[:, 0:512])
                    nc.tensor.matmul(sc_ps[:, 512:CH], lhsT=q16c[:, bsl],
                                     rhs=ct[:, 512:CH])
                    mae = poolD.tile([128, CH], BF16, tag="mae")
                    nc.scalar.activation(mae, sc_ps, AF.Sqrt,
                                         bias=maebias[:, 0:1], scale=1.0,
                                         accum_out=acc_mt[:, ci:ci + 1])
                    scr = poolD.tile([128, CH], BF16, tag="scr")
                    nc.vector.scalar_tensor_tensor(
                        out=scr, in0=labb, scalar=pseudo_i[:, bt:bt + 1],
                        in1=mae, op0=OP.is_equal, op1=OP.mult,
                        accum_out=acc_ms[:, ci:ci + 1])

            # =========== Final: fold chunk accumulators, write stats ========
            for bt in range(NBT):
                sl = slice(bt * NCH, (bt + 1) * NCH)
                nc.vector.tensor_reduce(stats_sb[:, bt:bt + 1], acc_ms[:, sl],
                                        axis=AX.X, op=OP.add)
                nc.vector.tensor_reduce(stats_sb[:, 8 + bt:9 + bt], acc_mt[:, sl],
                                        axis=AX.X, op=OP.add)
            nc.scalar.dma_start(out=stats.ap(), in_=stats_sb)

    nc.compile()
    return nc


_NC_CACHE = None


def _get_nc():
    global _NC_CACHE
    if _NC_CACHE is None:
        _NC_CACHE = build_kernel()
    return _NC_CACHE


def kernel(h_q, h_k, W_cls, b_cls, W_p1, b_p1, W_p2, b_p2,
           Wk_p1, bk_p1, Wk_p2, bk_p2, queue_emb, queue_emb_copy,
           info_label, coarse_labs):
    nc = _get_nc()
    import ml_dtypes
    bf16_t = ml_dtypes.bfloat16
    f16 = np.float16
    f32 = np.float32
    ca = np.ascontiguousarray

    h_q = np.asarray(h_q, f32)
    h_k = np.asarray(h_k, f32)
    queue_emb = np.asarray(queue_emb, f32)
    queue_emb_copy = np.asarray(queue_emb_copy, f32)
    info_label = np.asarray(info_label).astype(np.int32)
    coarse_labs = np.asarray(coarse_labs).astype(np.int32)

    w1 = ca(np.asarray(W_p1, f16))
    wk1 = ca(np.asarray(Wk_p1, f16))
    w2 = ca(np.asarray(W_p2, f16))
    wk2 = ca(np.asarray(Wk_p2, f16))
    wc = ca(np.asarray(W_cls, f16))
    bp1 = ca(np.asarray(b_p1, f16))
    bkp1 = ca(np.asarray(bk_p1, f16))
    bp2 = ca(np.asarray(b_p2, f32))
    bkp2 = ca(np.asarray(bk_p2, f32))
    bcv = ca(np.asarray(b_cls, f32))
    queueT = ca(queue_emb.T.astype(f16))        # [D, K]
    copyT = ca(queue_emb_copy.T.astype(f16))    # [D, K]

    in_maps = []
    for c in range(NCORES):
        bs = slice(c * BS, (c + 1) * BS)
        ks = slice(c * KS, (c + 1) * KS)
        in_maps.append({
            "hqT": ca(h_q[bs].T.astype(f16)),
            "hkT": ca(h_k[bs].T.astype(f16)),
            "w1": w1, "wk1": wk1, "w2": w2, "wk2": wk2, "wc": wc,
            "bp1": bp1, "bkp1": bkp1, "bp2": bp2, "bkp2": bkp2, "bcls": bcv,
            "qslT": ca(queueT[:, ks]),
            "cslT": ca(copyT[:, ks]),
            "cnat": ca(queue_emb_copy[ks].astype(bf16_t)),
            "labs": ca(info_label[ks].astype(f32)),
            "clab": ca(coarse_labs[bs].astype(f32)),
        })

    res = run_bass_kernel_spmd(nc, in_maps, list(range(NCORES)))
    results = res.results

    # ---- host-side gather / final scalar math ----
    logits = np.empty((B, 1 + K), f32)
    for c in range(NCORES):
        logits[:, 1 + c * KS: 1 + (c + 1) * KS] = results[c]["lneg"]
    stats = [results[c]["stats"] for c in range(NCORES)]
    for c in range(NCORES):
        logits[c * BS:(c + 1) * BS, 0] = stats[c][:, 25]

    # masked mae sums / total mae sums: partial over each core's K slice
    msum = np.zeros((B,), np.float64)
    mtot = np.zeros((B,), np.float64)
    for c in range(NCORES):
        msum += stats[c][:, 0:8].T.reshape(B).astype(np.float64)
        mtot += stats[c][:, 8:16].T.reshape(B).astype(np.float64)
    # pseudo labels (identical on every core; take core 0)
    pseudo = stats[0][:, 16:24].T.reshape(B).astype(np.int64)
    class_counts = np.bincount(info_label, minlength=NI)
    cnt = class_counts[pseudo].astype(np.float64)

    eps = 1e-6
    min_e = np.mean(msum / (cnt + eps))
    avg_inter = np.mean((mtot - msum) / (K - cnt + eps))
    dino_loss = np.float32(min_e + (2.0 - avg_inter))

    logp = np.concatenate([stats[c][:, 24] for c in range(NCORES)])
    cls_loss = np.float32(-logp.mean())

    labels = np.zeros((B,), np.int32)
    return logits, labels, dino_loss, cls_loss


# revision 12
# speedup vs baseline: 3.6651x; 1.1395x over previous
"""MoCo forward kernel for 8 Trainium2 NeuronCores.

Sharding:
  - Projections (h->q, h->k), l_pos, cls head: data-parallel over batch
    (128 rows per core), followed by an AllGather of raw q vectors.
  - Per-info-class centroid sums: sharded over the queue dim K
    (8192 rows per core) via a one-hot matmul, combined with an AllReduce.
  - The big B x K similarity / mask / MAE phase: sharded over K columns
    (each core computes all 1024 rows against its 8192 queue columns),
    which avoids replicating the 2x 32MB queue reads on every core.

All matmuls run in fp16 (same PE rate as bf16, 4x the mantissa) with fp32
PSUM accumulation; l2 normalization factors are folded into pre-scaled
copies of q so the inner loop is pure matmul + one ACT sqrt + one DVE
copy + one DVE masked-reduce + one GPSIMD compare per tile.
"""

import sys

sys.path.insert(0, '/opt/trn_rl_repo')

from contextlib import ExitStack

import numpy as np

import concourse.bass as bass
import concourse.mybir as mybir
import concourse.tile as tile
from concourse import bacc
from concourse.bass_utils import run_bass_kernel_spmd
from concourse.masks import make_identity

NCORES = 8
B, E, D, K = 1024, 2048, 128, 65536
NI, NCLS = 1024, 100
T = 0.07
BS = B // NCORES          # 128 batch rows per core
KS = K // NCORES          # 8192 queue rows/cols per core
CH = 1024                 # chunk width in the K loop
NCH = KS // CH            # 16 chunks per core
NBT = B // 128            # 8 batch tiles of 128
ET = E // 128             # 16 contraction tiles over EMBED

F16 = mybir.dt.float16
BF16 = mybir.dt.bfloat16
F32 = mybir.dt.float32
I32 = mybir.dt.int32
AX = mybir.AxisListType
OP = mybir.AluOpType
AF = mybir.ActivationFunctionType


def build_kernel():
    nc = bacc.Bacc('TRN2', target_bir_lowering=False, debug=False,
                   num_devices=NCORES)

    # ---- DRAM I/O ----
    hqT = nc.dram_tensor("hqT", [E, BS], F16, kind="ExternalInput")
    hkT = nc.dram_tensor("hkT", [E, BS], F16, kind="ExternalInput")
    w1 = nc.dram_tensor("w1", [E, E], F16, kind="ExternalInput")
    wk1 = nc.dram_tensor("wk1", [E, E], F16, kind="ExternalInput")
    w2 = nc.dram_tensor("w2", [E, D], F16, kind="ExternalInput")
    wk2 = nc.dram_tensor("wk2", [E, D], F16, kind="ExternalInput")
    wc = nc.dram_tensor("wc", [E, NCLS], F16, kind="ExternalInput")
    bp1 = nc.dram_tensor("bp1", [E], F16, kind="ExternalInput")
    bkp1 = nc.dram_tensor("bkp1", [E], F16, kind="ExternalInput")
    bp2 = nc.dram_tensor("bp2", [D], F32, kind="ExternalInput")
    bkp2 = nc.dram_tensor("bkp2", [D], F32, kind="ExternalInput")
    bcls = nc.dram_tensor("bcls", [NCLS], F32, kind="ExternalInput")
    qslT = nc.dram_tensor("qslT", [D, KS], F16, kind="ExternalInput")
    cslT = nc.dram_tensor("cslT", [D, KS], F16, kind="ExternalInput")
    cnat = nc.dram_tensor("cnat", [KS, D], BF16, kind="ExternalInput")
    labs = nc.dram_tensor("labs", [KS], F32, kind="ExternalInput")
    clab = nc.dram_tensor("clab", [BS], F32, kind="ExternalInput")

    lneg = nc.dram_tensor("lneg", [B, KS], F32, kind="ExternalOutput")
    stats = nc.dram_tensor("stats", [BS, 32], F32, kind="ExternalOutput")

    groups = [list(range(NCORES))]

    with tile.TileContext(nc) as tc, ExitStack() as ctx:
        consts = ctx.enter_context(tc.tile_pool(name="consts", bufs=1))
        persist = ctx.enter_context(tc.tile_pool(name="persist", bufs=1))
        dram = ctx.enter_context(tc.tile_pool(name="dram", bufs=1, space="DRAM"))

        # constants
        iota_i = consts.tile([128, NI], F32)
        nc.gpsimd.iota(iota_i, pattern=[[1, NI]], base=0, channel_multiplier=0,
                       allow_small_or_imprecise_dtypes=True)
        ones_col = consts.tile([128, 1], F32)
        nc.vector.memset(ones_col, 1.0)
        ones_row = consts.tile([1, 128], F32)
        nc.vector.memset(ones_row, 1.0)

        stats_sb = persist.tile([128, 32], F32)
        nc.vector.memset(stats_sb, 0.0)
        eps30 = consts.tile([1, 1], F32)
        nc.vector.memset(eps30, 1e-30)
        maebias = consts.tile([128, 1], F32)
        nc.vector.memset(maebias, 2.0 + 1e-6)

        # =========== Phase A: centroid partial sums over own K slice =========
        # cent_ps[d, i] += sum_k copy[k, d] * onehot[k, i]
        cent_sum = persist.tile([128, NI], F32)
        with tc.tile_pool(name="psA", bufs=1, space="PSUM") as psA, \
             tc.tile_pool(name="poolA", bufs=3) as poolA:
            cent_ps0 = psA.tile([128, 512], F32, tag="c0")
            cent_ps1 = psA.tile([128, 512], F32, tag="c1")
            nkt = KS // 128  # 64
            labs_sb = poolA.tile([128, nkt], F32, tag="labs_sb", bufs=1)
            nc.sync.dma_start(out=labs_sb, in_=labs.rearrange("(t p) -> p t", p=128))
            cn_sb = []
            for g in range(4):
                t = poolA.tile([128, 16, D], BF16, tag=f"cn{g}", bufs=1,
                               name=f"cn_sb{g}")
                nc.sync.dma_start(
                    out=t, in_=cnat[g * 2048:(g + 1) * 2048, :]
                    .rearrange("(t p) d -> p t d", p=128))
                cn_sb.append(t)
            for kt in range(nkt):
                oh = poolA.tile([128, NI], BF16, tag="oh")
                nc.vector.tensor_scalar(oh, iota_i, labs_sb[:, kt:kt + 1], None,
                                        OP.is_equal)
                cpt = cn_sb[kt // 16][:, kt % 16, :]
                nc.tensor.matmul(cent_ps0, lhsT=cpt, rhs=oh[:, 0:512],
                                 start=(kt == 0), stop=(kt == nkt - 1))
                nc.tensor.matmul(cent_ps1, lhsT=cpt, rhs=oh[:, 512:NI],
                                 start=(kt == 0), stop=(kt == nkt - 1))
            nc.scalar.copy(cent_sum[:, 0:512], cent_ps0)
            nc.scalar.copy(cent_sum[:, 512:NI], cent_ps1)

        cent_in = dram.tile([128, NI], F32)
        cent_out = dram.tile([128, NI], F32)
        nc.sync.dma_start(out=cent_in, in_=cent_sum)
        nc.gpsimd.collective_compute(
            "AllReduce", OP.add, replica_groups=groups,
            ins=[cent_in.opt()], outs=[cent_out.opt()])
        centall = persist.tile([128, NI], F32)
        nc.sync.dma_start(out=centall, in_=cent_out)

        # =========== Phase B: projections (own 128 batch rows) ==============
        # Layer 1 runs h-stationary (16 LDWs instead of 256): for each e-tile,
        # lhsT = hT tile, moving = a 2048-wide row block of W1. The bias is
        # seeded into PSUM with a ones-row matmul, giving z1 = relu(h@W1+b1)
        # in [b, oe] layout, which PE-transposes to [oe, b] for layer 2.
        hq_sb = persist.tile([128, ET, BS], F16)
        nc.sync.dma_start(out=hq_sb, in_=hqT.rearrange("(t p) b -> p t b", p=128))
        hk_sb = persist.tile([128, ET, BS], F16)
        nc.sync.dma_start(out=hk_sb, in_=hkT.rearrange("(t p) b -> p t b", p=128))
        w2_sb = persist.tile([128, ET, D], F16)
        nc.sync.dma_start(out=w2_sb, in_=w2.rearrange("(t p) d -> p t d", p=128))
        wk2_sb = persist.tile([128, ET, D], F16)
        nc.sync.dma_start(out=wk2_sb, in_=wk2.rearrange("(t p) d -> p t d", p=128))
        b1row_q = persist.tile([1, E], F16)
        nc.sync.dma_start(out=b1row_q, in_=bp1.rearrange("(o e) -> o e", o=1))
        b1row_k = persist.tile([1, E], F16)
        nc.sync.dma_start(out=b1row_k, in_=bkp1.rearrange("(o e) -> o e", o=1))
        bp2_sb = persist.tile([128, 1], F32)
        nc.sync.dma_start(out=bp2_sb, in_=bp2.rearrange("(p o) -> p o", o=1))
        bkp2_sb = persist.tile([128, 1], F32)
        nc.sync.dma_start(out=bkp2_sb, in_=bkp2.rearrange("(p o) -> p o", o=1))
        ones1 = consts.tile([1, 128], F16)
        nc.vector.memset(ones1, 1.0)
        ident = consts.tile([128, 128], F16)
        make_identity(nc, ident)

        qraw = persist.tile([128, BS], F32)   # [d, b] fp32, own shard
        kraw = persist.tile([128, BS], F32)
        ag_in = dram.tile([128, BS], F32)
        ag_out = dram.tile([NCORES, 128, BS], F32)

        with tc.tile_pool(name="psB", bufs=1, space="PSUM") as psB, \
             tc.tile_pool(name="psT", bufs=2, space="PSUM") as psT, \
             tc.tile_pool(name="poolB", bufs=3) as poolB, \
             tc.tile_pool(name="z1", bufs=1) as z1pool:
            ps_small = psB.tile([128, 16], F32, tag="small")
            zps = [psB.tile([128, 512], F32, tag=f"zc{c}", name=f"zps{c}") for c in range(4)]
            for branch in range(2):
                wsrc = w1 if branch == 0 else wk1
                hsb = hq_sb if branch == 0 else hk_sb
                b1r = b1row_q if branch == 0 else b1row_k
                w2sb = w2_sb if branch == 0 else wk2_sb
                b2sb = bp2_sb if branch == 0 else bkp2_sb
                zout = qraw if branch == 0 else kraw
                for c in range(4):
                    nc.tensor.matmul(zps[c], lhsT=ones1,
                                     rhs=b1r[:, c * 512:(c + 1) * 512],
                                     start=True, stop=False)
                for et in range(ET):
                    wrow = poolB.tile([128, E], F16, tag="wrow")
                    nc.sync.dma_start(out=wrow,
                                      in_=wsrc[et * 128:(et + 1) * 128, :])
                    for c in range(4):
                        nc.tensor.matmul(zps[c], lhsT=hsb[:, et, :],
                                         rhs=wrow[:, c * 512:(c + 1) * 512],
                                         start=False, stop=(et == ET - 1))
                z1f = z1pool.tile([128, E], F16, tag=f"z1f_{branch}")
                for c in range(4):
                    nc.scalar.activation(z1f[:, c * 512:(c + 1) * 512], zps[c],
                                         AF.Relu, bias=0.0, scale=1.0)
                z1t = z1pool.tile([128, ET, BS], F16, tag=f"z1t_{branch}")
                for ot in range(ET):
                    tp = psT.tile([128, 128], F16, tag="tp")
                    nc.tensor.transpose(tp, z1f[:, ot * 128:(ot + 1) * 128], ident)
                    nc.scalar.copy(z1t[:, ot, :], tp)
                qps = psB.tile([128, BS], F32, tag="qps")
                for et in range(ET):
                    nc.tensor.matmul(qps, lhsT=w2sb[:, et, :], rhs=z1t[:, et, :],
                                     start=(et == 0), stop=(et == ET - 1))
                nc.scalar.activation(zout, qps, AF.Identity,
                                     bias=b2sb[:, 0:1], scale=1.0)
                if branch == 0:
                    # AllGather of raw q dispatched while the k branch computes
                    nc.sync.dma_start(out=ag_in, in_=qraw)
                    nc.gpsimd.collective_compute(
                        "AllGather", OP.bypass, replica_groups=groups,
                        ins=[ag_in.opt()], outs=[ag_out.opt()])

            # own-shard norms and l_pos
            sq = poolB.tile([128, BS], F32, tag="sqloc")
            nc.vector.tensor_mul(sq, qraw, qraw)
            nc.tensor.matmul(ps_small[:, 8:9], lhsT=sq, rhs=ones_col)
            sk = poolB.tile([128, BS], F32, tag="sqloc")
            nc.vector.tensor_mul(sk, kraw, kraw)
            nc.tensor.matmul(ps_small[:, 9:10], lhsT=sk, rhs=ones_col)
            pq = poolB.tile([128, BS], F32, tag="sqloc")
            nc.vector.tensor_mul(pq, qraw, kraw)
            nc.tensor.matmul(ps_small[:, 10:11], lhsT=pq, rhs=ones_col)
            nrm2 = persist.tile([128, 4], F32)
            nc.scalar.activation(nrm2[:, 0:2], ps_small[:, 8:10], AF.Sqrt,
                                 bias=0.0, scale=1.0)
            rloc = persist.tile([128, 2], F32)
            nc.vector.reciprocal(rloc, nrm2[:, 0:2])
            # l_pos = (q.k) * rnq * rnk / T  -> stats col 25
            lposv = persist.tile([128, 1], F32)
            nc.vector.tensor_scalar(lposv, ps_small[:, 10:11],
                                    rloc[:, 0:1], rloc[:, 1:2], OP.mult, OP.mult)
            nc.vector.tensor_scalar_mul(stats_sb[:, 25:26], lposv, 1.0 / T)

        qall = persist.tile([128, B], F32)
        for c in range(NCORES):
            nc.sync.dma_start(out=qall[:, c * BS:(c + 1) * BS], in_=ag_out[c])

        # global norms (row layout) and pre-scaled fp16 copies of q
        q16q = persist.tile([128, B], F16)   # q * rnq / T      (logits)
        q16c = persist.tile([128, B], F16)   # q * (-2 rnq)     (mae)
        with tc.tile_pool(name="psC", bufs=2, space="PSUM") as psC, \
             tc.tile_pool(name="poolC", bufs=2) as poolC:
            sqall = poolC.tile([128, B], F32, tag="sqall")
            nc.vector.tensor_mul(sqall, qall, qall)
            nrow_ps = psC.tile([1, B], F32, tag="nrow")
            nc.tensor.matmul(nrow_ps[:, 0:512], lhsT=ones_col, rhs=sqall[:, 0:512])
            nc.tensor.matmul(nrow_ps[:, 512:B], lhsT=ones_col, rhs=sqall[:, 512:B])
            nrow = poolC.tile([1, B], F32, tag="nrowsb")
            nc.scalar.activation(nrow[:, 0:512], nrow_ps[:, 0:512], AF.Sqrt,
                                 bias=0.0, scale=1.0)
            nc.scalar.activation(nrow[:, 512:B], nrow_ps[:, 512:B], AF.Sqrt,
                                 bias=0.0, scale=1.0)
            for half in range(2):
                sl = slice(half * 512, (half + 1) * 512)
                bq_ps = psC.tile([128, 512], F32, tag="bq")
                nc.tensor.matmul(bq_ps, lhsT=ones_row, rhs=nrow[:, sl])
                rbc = poolC.tile([128, 512], F32, tag="rbc")
                nc.vector.reciprocal(rbc, bq_ps)
                nc.vector.scalar_tensor_tensor(
                    out=q16q[:, sl], in0=qall[:, sl], scalar=1.0 / T, in1=rbc,
                    op0=OP.mult, op1=OP.mult)
                nc.vector.scalar_tensor_tensor(
                    out=q16c[:, sl], in0=qall[:, sl], scalar=-2.0, in1=rbc,
                    op0=OP.mult, op1=OP.mult)

            # ========= Phase C: centroid normalize + pseudo labels ==========
            sqc = poolC.tile([128, NI], F32, tag="sqall")
            nc.vector.tensor_mul(sqc, centall, centall)
            cn_ps = psC.tile([1, NI], F32, tag="nrow")
            nc.tensor.matmul(cn_ps[:, 0:512], lhsT=ones_col, rhs=sqc[:, 0:512])
            nc.tensor.matmul(cn_ps[:, 512:NI], lhsT=ones_col, rhs=sqc[:, 512:NI])
            cnn = poolC.tile([1, NI], F32, tag="nrowsb")
            nc.scalar.activation(cnn[:, 0:512], cn_ps[:, 0:512], AF.Sqrt,
                                 bias=eps30[0:1, 0:1], scale=1.0)
            nc.scalar.activation(cnn[:, 512:NI], cn_ps[:, 512:NI], AF.Sqrt,
                                 bias=eps30[0:1, 0:1], scale=1.0)
            centn = persist.tile([128, NI], F16)
            for half in range(2):
                sl = slice(half * 512, (half + 1) * 512)
                cb_ps = psC.tile([128, 512], F32, tag="bq")
                nc.tensor.matmul(cb_ps, lhsT=ones_row, rhs=cnn[:, sl])
                rcb = poolC.tile([128, 512], F32, tag="rbc")
                nc.vector.reciprocal(rcb, cb_ps)
                nc.vector.tensor_mul(centn[:, sl], centall[:, sl], rcb)

            pseudo_i = persist.tile([128, NBT], F32)
            for bt in range(NBT):
                plog = poolC.tile([128, NI], F32, tag="plog")
                for half in range(2):
                    sl = slice(half * 512, (half + 1) * 512)
                    pl_ps = psC.tile([128, 512], F32, tag="bq")
                    nc.tensor.matmul(pl_ps, lhsT=q16q[:, bt * 128:(bt + 1) * 128],
                                     rhs=centn[:, sl])
                    nc.scalar.copy(plog[:, sl], pl_ps)
                mx8 = poolC.tile([128, 8], F32, tag="mx8")
                ix8 = poolC.tile([128, 8], mybir.dt.uint32, tag="ix8")
                nc.vector.max_with_indices(mx8, ix8, plog)
                nc.vector.tensor_copy(pseudo_i[:, bt:bt + 1], ix8[:, 0:1])
            # export pseudo (as f32) for host-side exact mask counts
            nc.vector.tensor_copy(stats_sb[:, 16:24], pseudo_i)

        # =========== Phase E: classification head (own rows) ============
        wc_sb = persist.tile([128, ET, NCLS], F16)
        nc.sync.dma_start(out=wc_sb, in_=wc.rearrange("(t p) c -> p t c", p=128))
        bcls_bc = persist.tile([128, NCLS], F32)
        nc.gpsimd.dma_start(
            out=bcls_bc,
            in_=bass.AP(tensor=bcls.ap().tensor, offset=0,
                        ap=[[0, 128]] + list(bcls.ap().ap)))
        clab_sb = persist.tile([128, 1], F32)
        nc.sync.dma_start(out=clab_sb, in_=clab.rearrange("(p o) -> p o", o=1))
        with tc.tile_pool(name="psE", bufs=1, space="PSUM") as psE, \
             tc.tile_pool(name="poolE", bufs=1) as poolE:
            cls_ps = psE.tile([128, NCLS], F32, tag="cls")
            for et in range(ET):
                nc.tensor.matmul(cls_ps, lhsT=hq_sb[:, et, :], rhs=wc_sb[:, et, :],
                                 start=(et == 0), stop=(et == ET - 1))
            coarse = poolE.tile([128, NCLS], F32, tag="coarse")
            nc.vector.tensor_add(coarse, cls_ps, bcls_bc)
            mxc = poolE.tile([128, 1], F32, tag="mxc")
            nc.vector.tensor_reduce(mxc, coarse, axis=AX.X, op=OP.max)
            mxn = poolE.tile([128, 1], F32, tag="mxn")
            nc.vector.tensor_scalar_mul(mxn, mxc, -1.0)
            es = poolE.tile([128, NCLS], BF16, tag="es")
            sume = poolE.tile([128, 1], F32, tag="sume")
            nc.scalar.activation(es, coarse, AF.Exp, bias=mxn[:, 0:1], scale=1.0,
                                 accum_out=sume)
            lse = poolE.tile([128, 1], F32, tag="lse")
            nc.scalar.activation(lse, sume, AF.Ln, bias=0.0, scale=1.0)
            ohc = poolE.tile([128, NCLS], F32, tag="ohc")
            nc.vector.tensor_scalar(ohc, iota_i[:, 0:NCLS], clab_sb, None,
                                    OP.is_equal)
            scrc = poolE.tile([128, NCLS], F32, tag="ohscr")
            picked = poolE.tile([128, 1], F32, tag="picked")
            nc.vector.tensor_mul(scrc, coarse, ohc)
            nc.vector.tensor_reduce(picked, scrc, axis=AX.X, op=OP.add)
            t1 = poolE.tile([128, 1], F32, tag="t1")
            nc.vector.tensor_sub(t1, picked, mxc)
            nc.vector.tensor_sub(stats_sb[:, 24:25], t1, lse)

        # =========== Phase D: the B x KS similarity / mae / mask loop =======
        acc_ms = persist.tile([128, NBT * NCH], F32)   # masked mae sums
        acc_mt = persist.tile([128, NBT * NCH], F32)   # total mae sums
        with tc.tile_pool(name="psD", bufs=2, space="PSUM") as psD, \
             tc.tile_pool(name="poolD", bufs=4) as poolD, \
             tc.tile_pool(name="outD", bufs=4) as outD:
            for ch in range(NCH):
                c0 = ch * CH
                qt = poolD.tile([128, CH], F16, tag="qt")
                nc.sync.dma_start(out=qt, in_=qslT[:, c0:c0 + CH])
                ct = poolD.tile([128, CH], F16, tag="ct")
                nc.sync.dma_start(out=ct, in_=cslT[:, c0:c0 + CH])
                labb = poolD.tile([128, CH], F32, tag="labb")
                lab_sl = labs[c0:c0 + CH]
                nc.gpsimd.dma_start(
                    out=labb,
                    in_=bass.AP(tensor=lab_sl.tensor, offset=lab_sl.offset,
                                ap=[[0, 128]] + list(lab_sl.ap)))
                for bt in range(NBT):
                    bsl = slice(bt * 128, (bt + 1) * 128)
                    ci = bt * NCH + ch
                    sq_ps = psD.tile([128, CH], F32, tag="sq")
                    nc.tensor.matmul(sq_ps[:, 0:512], lhsT=q16q[:, bsl],
                                     rhs=qt[:, 0:512])
                    nc.tensor.matmul(sq_ps[:, 512:CH], lhsT=q16q[:, bsl],
                                     rhs=qt[:, 512:CH])
                    lgch = outD.tile([128, CH], F32, tag="lgch")
                    if (ch * NBT + bt) % 2 == 0:
                        nc.vector.tensor_copy(lgch, sq_ps)
                    else:
                        nc.scalar.copy(lgch, sq_ps)
                    nc.sync.dma_start(out=lneg[bsl, c0:c0 + CH], in_=lgch)
                    sc_ps = psD.tile([128, CH], F32, tag="sc")
                    nc.tensor.matmul(sc_ps[:, 0:512], lhsT=q16c[:, bsl],
                                     rhs=ct[:, 0:512])
                    nc.tensor.matmul(sc_ps[:, 512:CH], lhsT=q16c[:, bsl],
                                     rhs=ct[:, 512:CH])
                    mae = poolD.tile([128, CH], BF16, tag="mae")
                    nc.scalar.activation(mae, sc_ps, AF.Sqrt,
                                         bias=maebias[:, 0:1], scale=1.0,
                                         accum_out=acc_mt[:, ci:ci + 1])
                    scr = poolD.tile([128, CH], BF16, tag="scr")
                    nc.vector.scalar_tensor_tensor(
                        out=scr, in0=labb, scalar=pseudo_i[:, bt:bt + 1],
                        in1=mae, op0=OP.is_equal, op1=OP.mult,
                        accum_out=acc_ms[:, ci:ci + 1])

            # =========== Final: fold chunk accumulators, write stats ========
            for bt in range(NBT):
                sl = slice(bt * NCH, (bt + 1) * NCH)
                nc.vector.tensor_reduce(stats_sb[:, bt:bt + 1], acc_ms[:, sl],
                                        axis=AX.X, op=OP.add)
                nc.vector.tensor_reduce(stats_sb[:, 8 + bt:9 + bt], acc_mt[:, sl],
                                        axis=AX.X, op=OP.add)
            nc.scalar.dma_start(out=stats.ap(), in_=stats_sb)

    nc.compile()
    return nc


_NC_CACHE = None


def _get_nc():
    global _NC_CACHE
    if _NC_CACHE is None:
        _NC_CACHE = build_kernel()
    return _NC_CACHE


def kernel(h_q, h_k, W_cls, b_cls, W_p1, b_p1, W_p2, b_p2,
           Wk_p1, bk_p1, Wk_p2, bk_p2, queue_emb, queue_emb_copy,
           info_label, coarse_labs):
    nc = _get_nc()
    import ml_dtypes
    bf16_t = ml_dtypes.bfloat16
    f16 = np.float16
    f32 = np.float32
    ca = np.ascontiguousarray

    h_q = np.asarray(h_q, f32)
    h_k = np.asarray(h_k, f32)
    queue_emb = np.asarray(queue_emb, f32)
    queue_emb_copy = np.asarray(queue_emb_copy, f32)
    info_label = np.asarray(info_label).astype(np.int32)
    coarse_labs = np.asarray(coarse_labs).astype(np.int32)

    w1 = ca(np.asarray(W_p1, f16))
    wk1 = ca(np.asarray(Wk_p1, f16))
    w2 = ca(np.asarray(W_p2, f16))
    wk2 = ca(np.asarray(Wk_p2, f16))
    wc = ca(np.asarray(W_cls, f16))
    bp1 = ca(np.asarray(b_p1, f16))
    bkp1 = ca(np.asarray(bk_p1, f16))
    bp2 = ca(np.asarray(b_p2, f32))
    bkp2 = ca(np.asarray(bk_p2, f32))
    bcv = ca(np.asarray(b_cls, f32))
    queueT = ca(queue_emb.T.astype(f16))        # [D, K]
    copyT = ca(queue_emb_copy.T.astype(f16))    # [D, K]

    in_maps = []
    for c in range(NCORES):
        bs = slice(c * BS, (c + 1) * BS)
        ks = slice(c * KS, (c + 1) * KS)
        in_maps.append({
            "hqT": ca(h_q[bs].T.astype(f16)),
            "hkT": ca(h_k[bs].T.astype(f16)),
            "w1": w1, "wk1": wk1, "w2": w2, "wk2": wk2, "wc": wc,
            "bp1": bp1, "bkp1": bkp1, "bp2": bp2, "bkp2": bkp2, "bcls": bcv,
            "qslT": ca(queueT[:, ks]),
            "cslT": ca(copyT[:, ks]),
            "cnat": ca(queue_emb_copy[ks].astype(bf16_t)),
            "labs": ca(info_label[ks].astype(f32)),
            "clab": ca(coarse_labs[bs].astype(f32)),
        })

    res = run_bass_kernel_spmd(nc, in_maps, list(range(NCORES)))
    results = res.results

    # ---- host-side gather / final scalar math ----
    logits = np.empty((B, 1 + K), f32)
    for c in range(NCORES):
        logits[:, 1 + c * KS: 1 + (c + 1) * KS] = results[c]["lneg"]
    stats = [results[c]["stats"] for c in range(NCORES)]
    for c in range(NCORES):
        logits[c * BS:(c + 1) * BS, 0] = stats[c][:, 25]

    # masked mae sums / total mae sums: partial over each core's K slice
    msum = np.zeros((B,), np.float64)
    mtot = np.zeros((B,), np.float64)
    for c in range(NCORES):
        msum += stats[c][:, 0:8].T.reshape(B).astype(np.float64)
        mtot += stats[c][:, 8:16].T.reshape(B).astype(np.float64)
    # pseudo labels (identical on every core; take core 0)
    pseudo = stats[0][:, 16:24].T.reshape(B).astype(np.int64)
    class_counts = np.bincount(info_label, minlength=NI)
    cnt = class_counts[pseudo].astype(np.float64)

    eps = 1e-6
    min_e = np.mean(msum / (cnt + eps))
    avg_inter = np.mean((mtot - msum) / (K - cnt + eps))
    dino_loss = np.float32(min_e + (2.0 - avg_inter))

    logp = np.concatenate([stats[c][:, 24] for c in range(NCORES)])
    cls_loss = np.float32(-logp.mean())

    labels = np.zeros((B,), np.int32)
    return logits, labels, dino_loss, cls_loss


# revision 13
# speedup vs baseline: 3.8091x; 1.0393x over previous
"""MoCo forward kernel for 8 Trainium2 NeuronCores.

Sharding:
  - Projections (h->q, h->k), l_pos, cls head: data-parallel over batch
    (128 rows per core), followed by an AllGather of raw q vectors.
  - Per-info-class centroid sums: sharded over the queue dim K
    (8192 rows per core) via a one-hot matmul, combined with an AllReduce.
  - The big B x K similarity / mask / MAE phase: sharded over K columns
    (each core computes all 1024 rows against its 8192 queue columns),
    which avoids replicating the 2x 32MB queue reads on every core.

All matmuls run in fp16 (same PE rate as bf16, 4x the mantissa) with fp32
PSUM accumulation; l2 normalization factors are folded into pre-scaled
copies of q so the inner loop is pure matmul + one ACT sqrt + one DVE
copy + one DVE masked-reduce + one GPSIMD compare per tile.
"""

import sys

sys.path.insert(0, '/opt/trn_rl_repo')

from contextlib import ExitStack

import numpy as np

import concourse.bass as bass
import concourse.mybir as mybir
import concourse.tile as tile
from concourse import bacc
from concourse.bass_utils import run_bass_kernel_spmd
from concourse.masks import make_identity

NCORES = 8
B, E, D, K = 1024, 2048, 128, 65536
NI, NCLS = 1024, 100
T = 0.07
BS = B // NCORES          # 128 batch rows per core
KS = K // NCORES          # 8192 queue rows/cols per core
CH = 1024                 # chunk width in the K loop
NCH = KS // CH            # 16 chunks per core
NBT = B // 128            # 8 batch tiles of 128
ET = E // 128             # 16 contraction tiles over EMBED

F16 = mybir.dt.float16
BF16 = mybir.dt.bfloat16
F32 = mybir.dt.float32
I32 = mybir.dt.int32
AX = mybir.AxisListType
OP = mybir.AluOpType
AF = mybir.ActivationFunctionType


def build_kernel():
    nc = bacc.Bacc('TRN2', target_bir_lowering=False, debug=False,
                   num_devices=NCORES)

    # ---- DRAM I/O ----
    hqT = nc.dram_tensor("hqT", [E, BS], F16, kind="ExternalInput")
    hkT = nc.dram_tensor("hkT", [E, BS], F16, kind="ExternalInput")
    w1 = nc.dram_tensor("w1", [E, E], F16, kind="ExternalInput")
    wk1 = nc.dram_tensor("wk1", [E, E], F16, kind="ExternalInput")
    w2 = nc.dram_tensor("w2", [E, D], F16, kind="ExternalInput")
    wk2 = nc.dram_tensor("wk2", [E, D], F16, kind="ExternalInput")
    wc = nc.dram_tensor("wc", [E, NCLS], F16, kind="ExternalInput")
    bp1 = nc.dram_tensor("bp1", [E], F16, kind="ExternalInput")
    bkp1 = nc.dram_tensor("bkp1", [E], F16, kind="ExternalInput")
    bp2 = nc.dram_tensor("bp2", [D], F32, kind="ExternalInput")
    bkp2 = nc.dram_tensor("bkp2", [D], F32, kind="ExternalInput")
    bcls = nc.dram_tensor("bcls", [NCLS], F32, kind="ExternalInput")
    qslT = nc.dram_tensor("qslT", [D, KS], F16, kind="ExternalInput")
    cslT = nc.dram_tensor("cslT", [D, KS], F16, kind="ExternalInput")
    cnat = nc.dram_tensor("cnat", [KS, D], BF16, kind="ExternalInput")
    labs = nc.dram_tensor("labs", [KS], F32, kind="ExternalInput")
    clab = nc.dram_tensor("clab", [BS], F32, kind="ExternalInput")

    lneg = nc.dram_tensor("lneg", [B, KS], F32, kind="ExternalOutput")
    stats = nc.dram_tensor("stats", [BS, 32], F32, kind="ExternalOutput")

    groups = [list(range(NCORES))]

    with tile.TileContext(nc) as tc, ExitStack() as ctx:
        consts = ctx.enter_context(tc.tile_pool(name="consts", bufs=1))
        persist = ctx.enter_context(tc.tile_pool(name="persist", bufs=1))
        dram = ctx.enter_context(tc.tile_pool(name="dram", bufs=1, space="DRAM"))

        # constants
        iota_i = consts.tile([128, NI], F32)
        nc.gpsimd.iota(iota_i, pattern=[[1, NI]], base=0, channel_multiplier=0,
                       allow_small_or_imprecise_dtypes=True)
        ones_col = consts.tile([128, 1], F32)
        nc.vector.memset(ones_col, 1.0)
        ones_row = consts.tile([1, 128], F32)
        nc.vector.memset(ones_row, 1.0)

        stats_sb = persist.tile([128, 32], F32)
        nc.vector.memset(stats_sb, 0.0)
        eps30 = consts.tile([1, 1], F32)
        nc.vector.memset(eps30, 1e-30)
        maebias = consts.tile([128, 1], F32)
        nc.vector.memset(maebias, 2.0 + 1e-6)

        # =========== Phase A: centroid partial sums over own K slice =========
        # cent_ps[d, i] += sum_k copy[k, d] * onehot[k, i]
        cent_sum = persist.tile([128, NI], F32)
        with tc.tile_pool(name="psA", bufs=1, space="PSUM") as psA, \
             tc.tile_pool(name="poolA", bufs=3) as poolA:
            cent_ps0 = psA.tile([128, 512], F32, tag="c0")
            cent_ps1 = psA.tile([128, 512], F32, tag="c1")
            nkt = KS // 128  # 64
            labs_sb = poolA.tile([128, nkt], F32, tag="labs_sb", bufs=1)
            nc.sync.dma_start(out=labs_sb, in_=labs.rearrange("(t p) -> p t", p=128))
            cn_sb = []
            for g in range(4):
                t = poolA.tile([128, 16, D], BF16, tag=f"cn{g}", bufs=1,
                               name=f"cn_sb{g}")
                nc.sync.dma_start(
                    out=t, in_=cnat[g * 2048:(g + 1) * 2048, :]
                    .rearrange("(t p) d -> p t d", p=128))
                cn_sb.append(t)
            for kt in range(nkt):
                oh = poolA.tile([128, NI], BF16, tag="oh")
                nc.vector.tensor_scalar(oh, iota_i, labs_sb[:, kt:kt + 1], None,
                                        OP.is_equal)
                cpt = cn_sb[kt // 16][:, kt % 16, :]
                nc.tensor.matmul(cent_ps0, lhsT=cpt, rhs=oh[:, 0:512],
                                 start=(kt == 0), stop=(kt == nkt - 1))
                nc.tensor.matmul(cent_ps1, lhsT=cpt, rhs=oh[:, 512:NI],
                                 start=(kt == 0), stop=(kt == nkt - 1))
            nc.scalar.copy(cent_sum[:, 0:512], cent_ps0)
            nc.scalar.copy(cent_sum[:, 512:NI], cent_ps1)

        cent_in = dram.tile([128, NI], F32)
        cent_out = dram.tile([128, NI], F32)
        nc.gpsimd.dma_start(out=cent_in, in_=cent_sum)
        nc.gpsimd.collective_compute(
            "AllReduce", OP.add, replica_groups=groups,
            ins=[cent_in.opt()], outs=[cent_out.opt()])
        centall = persist.tile([128, NI], F32)
        nc.gpsimd.dma_start(out=centall, in_=cent_out)

        # =========== Phase B: projections (own 128 batch rows) ==============
        # Layer 1 runs h-stationary (16 LDWs instead of 256): for each e-tile,
        # lhsT = hT tile, moving = a 2048-wide row block of W1. The bias is
        # seeded into PSUM with a ones-row matmul, giving z1 = relu(h@W1+b1)
        # in [b, oe] layout, which PE-transposes to [oe, b] for layer 2.
        hq_sb = persist.tile([128, ET, BS], F16)
        nc.sync.dma_start(out=hq_sb, in_=hqT.rearrange("(t p) b -> p t b", p=128))
        hk_sb = persist.tile([128, ET, BS], F16)
        nc.sync.dma_start(out=hk_sb, in_=hkT.rearrange("(t p) b -> p t b", p=128))
        w2_sb = persist.tile([128, ET, D], F16)
        nc.sync.dma_start(out=w2_sb, in_=w2.rearrange("(t p) d -> p t d", p=128))
        wk2_sb = persist.tile([128, ET, D], F16)
        nc.sync.dma_start(out=wk2_sb, in_=wk2.rearrange("(t p) d -> p t d", p=128))
        b1row_q = persist.tile([1, E], F16)
        nc.sync.dma_start(out=b1row_q, in_=bp1.rearrange("(o e) -> o e", o=1))
        b1row_k = persist.tile([1, E], F16)
        nc.sync.dma_start(out=b1row_k, in_=bkp1.rearrange("(o e) -> o e", o=1))
        bp2_sb = persist.tile([128, 1], F32)
        nc.sync.dma_start(out=bp2_sb, in_=bp2.rearrange("(p o) -> p o", o=1))
        bkp2_sb = persist.tile([128, 1], F32)
        nc.sync.dma_start(out=bkp2_sb, in_=bkp2.rearrange("(p o) -> p o", o=1))
        ones1 = consts.tile([1, 128], F16)
        nc.vector.memset(ones1, 1.0)
        ident = consts.tile([128, 128], F16)
        make_identity(nc, ident)

        qraw = persist.tile([128, BS], F32)   # [d, b] fp32, own shard
        kraw = persist.tile([128, BS], F32)
        ag_in = dram.tile([128, BS], F32)
        ag_out = dram.tile([NCORES, 128, BS], F32)

        with tc.tile_pool(name="psB", bufs=1, space="PSUM") as psB, \
             tc.tile_pool(name="psT", bufs=2, space="PSUM") as psT, \
             tc.tile_pool(name="poolB", bufs=3) as poolB, \
             tc.tile_pool(name="z1", bufs=1) as z1pool:
            ps_small = psB.tile([128, 16], F32, tag="small")
            zps = [psB.tile([128, 512], F32, tag=f"zc{c}", name=f"zps{c}") for c in range(4)]
            for branch in range(2):
                wsrc = w1 if branch == 0 else wk1
                hsb = hq_sb if branch == 0 else hk_sb
                b1r = b1row_q if branch == 0 else b1row_k
                w2sb = w2_sb if branch == 0 else wk2_sb
                b2sb = bp2_sb if branch == 0 else bkp2_sb
                zout = qraw if branch == 0 else kraw
                for c in range(4):
                    nc.tensor.matmul(zps[c], lhsT=ones1,
                                     rhs=b1r[:, c * 512:(c + 1) * 512],
                                     start=True, stop=False)
                for et in range(ET):
                    wrow = poolB.tile([128, E], F16, tag="wrow")
                    weng = nc.sync if et % 2 == 0 else nc.scalar
                    weng.dma_start(out=wrow,
                                   in_=wsrc[et * 128:(et + 1) * 128, :])
                    for c in range(4):
                        nc.tensor.matmul(zps[c], lhsT=hsb[:, et, :],
                                         rhs=wrow[:, c * 512:(c + 1) * 512],
                                         start=False, stop=(et == ET - 1))
                z1f = z1pool.tile([128, E], F16, tag=f"z1f_{branch}")
                for c in range(4):
                    nc.scalar.activation(z1f[:, c * 512:(c + 1) * 512], zps[c],
                                         AF.Relu, bias=0.0, scale=1.0)
                z1t = z1pool.tile([128, ET, BS], F16, tag=f"z1t_{branch}")
                for og in range(ET // 4):
                    tp = psT.tile([128, 4, 128], F16, tag="tp")
                    for j in range(4):
                        ot = og * 4 + j
                        nc.tensor.transpose(tp[:, j, :],
                                            z1f[:, ot * 128:(ot + 1) * 128], ident)
                    nc.scalar.copy(z1t[:, og * 4:(og + 1) * 4, :], tp)
                qps = psB.tile([128, BS], F32, tag="qps")
                for et in range(ET):
                    nc.tensor.matmul(qps, lhsT=w2sb[:, et, :], rhs=z1t[:, et, :],
                                     start=(et == 0), stop=(et == ET - 1))
                nc.scalar.activation(zout, qps, AF.Identity,
                                     bias=b2sb[:, 0:1], scale=1.0)
                if branch == 0:
                    # AllGather of raw q dispatched while the k branch computes
                    nc.sync.dma_start(out=ag_in, in_=qraw)
                    nc.gpsimd.collective_compute(
                        "AllGather", OP.bypass, replica_groups=groups,
                        ins=[ag_in.opt()], outs=[ag_out.opt()])

            # own-shard norms and l_pos
            sq = poolB.tile([128, BS], F32, tag="sqloc")
            nc.vector.tensor_mul(sq, qraw, qraw)
            nc.tensor.matmul(ps_small[:, 8:9], lhsT=sq, rhs=ones_col)
            sk = poolB.tile([128, BS], F32, tag="sqloc")
            nc.vector.tensor_mul(sk, kraw, kraw)
            nc.tensor.matmul(ps_small[:, 9:10], lhsT=sk, rhs=ones_col)
            pq = poolB.tile([128, BS], F32, tag="sqloc")
            nc.vector.tensor_mul(pq, qraw, kraw)
            nc.tensor.matmul(ps_small[:, 10:11], lhsT=pq, rhs=ones_col)
            nrm2 = persist.tile([128, 4], F32)
            nc.scalar.activation(nrm2[:, 0:2], ps_small[:, 8:10], AF.Sqrt,
                                 bias=0.0, scale=1.0)
            rloc = persist.tile([128, 2], F32)
            nc.vector.reciprocal(rloc, nrm2[:, 0:2])
            # l_pos = (q.k) * rnq * rnk / T  -> stats col 25
            lposv = persist.tile([128, 1], F32)
            nc.vector.tensor_scalar(lposv, ps_small[:, 10:11],
                                    rloc[:, 0:1], rloc[:, 1:2], OP.mult, OP.mult)
            nc.vector.tensor_scalar_mul(stats_sb[:, 25:26], lposv, 1.0 / T)

        qall = persist.tile([128, B], F32)
        for c in range(NCORES):
            nc.sync.dma_start(out=qall[:, c * BS:(c + 1) * BS], in_=ag_out[c])

        # global norms (row layout) and pre-scaled fp16 copies of q
        q16q = persist.tile([128, B], F16)   # q * rnq / T      (logits)
        q16c = persist.tile([128, B], F16)   # q * (-2 rnq)     (mae)
        with tc.tile_pool(name="psC", bufs=2, space="PSUM") as psC, \
             tc.tile_pool(name="poolC", bufs=2) as poolC:
            sqall = poolC.tile([128, B], F32, tag="sqall")
            nc.vector.tensor_mul(sqall, qall, qall)
            nrow_ps = psC.tile([1, B], F32, tag="nrow")
            nc.tensor.matmul(nrow_ps[:, 0:512], lhsT=ones_col, rhs=sqall[:, 0:512])
            nc.tensor.matmul(nrow_ps[:, 512:B], lhsT=ones_col, rhs=sqall[:, 512:B])
            nrow = poolC.tile([1, B], F32, tag="nrowsb")
            nc.scalar.activation(nrow[:, 0:512], nrow_ps[:, 0:512], AF.Sqrt,
                                 bias=0.0, scale=1.0)
            nc.scalar.activation(nrow[:, 512:B], nrow_ps[:, 512:B], AF.Sqrt,
                                 bias=0.0, scale=1.0)
            for half in range(2):
                sl = slice(half * 512, (half + 1) * 512)
                bq_ps = psC.tile([128, 512], F32, tag="bq")
                nc.tensor.matmul(bq_ps, lhsT=ones_row, rhs=nrow[:, sl])
                rbc = poolC.tile([128, 512], F32, tag="rbc")
                nc.vector.reciprocal(rbc, bq_ps)
                nc.vector.scalar_tensor_tensor(
                    out=q16q[:, sl], in0=qall[:, sl], scalar=1.0 / T, in1=rbc,
                    op0=OP.mult, op1=OP.mult)
                nc.vector.scalar_tensor_tensor(
                    out=q16c[:, sl], in0=qall[:, sl], scalar=-2.0, in1=rbc,
                    op0=OP.mult, op1=OP.mult)

            # ========= Phase C: centroid normalize + pseudo labels ==========
            sqc = poolC.tile([128, NI], F32, tag="sqall")
            nc.vector.tensor_mul(sqc, centall, centall)
            cn_ps = psC.tile([1, NI], F32, tag="nrow")
            nc.tensor.matmul(cn_ps[:, 0:512], lhsT=ones_col, rhs=sqc[:, 0:512])
            nc.tensor.matmul(cn_ps[:, 512:NI], lhsT=ones_col, rhs=sqc[:, 512:NI])
            cnn = poolC.tile([1, NI], F32, tag="nrowsb")
            nc.scalar.activation(cnn[:, 0:512], cn_ps[:, 0:512], AF.Sqrt,
                                 bias=eps30[0:1, 0:1], scale=1.0)
            nc.scalar.activation(cnn[:, 512:NI], cn_ps[:, 512:NI], AF.Sqrt,
                                 bias=eps30[0:1, 0:1], scale=1.0)
            centn = persist.tile([128, NI], F16)
            for half in range(2):
                sl = slice(half * 512, (half + 1) * 512)
                cb_ps = psC.tile([128, 512], F32, tag="bq")
                nc.tensor.matmul(cb_ps, lhsT=ones_row, rhs=cnn[:, sl])
                rcb = poolC.tile([128, 512], F32, tag="rbc")
                nc.vector.reciprocal(rcb, cb_ps)
                nc.vector.tensor_mul(centn[:, sl], centall[:, sl], rcb)

            pseudo_i = persist.tile([128, NBT], F32)
            for bt in range(NBT):
                plog = poolC.tile([128, NI], F32, tag="plog")
                for half in range(2):
                    sl = slice(half * 512, (half + 1) * 512)
                    pl_ps = psC.tile([128, 512], F32, tag="bq")
                    nc.tensor.matmul(pl_ps, lhsT=q16q[:, bt * 128:(bt + 1) * 128],
                                     rhs=centn[:, sl])
                    if (bt + half) % 2 == 0:
                        nc.scalar.copy(plog[:, sl], pl_ps)
                    else:
                        nc.vector.tensor_copy(plog[:, sl], pl_ps)
                mx8 = poolC.tile([128, 8], F32, tag="mx8")
                ix8 = poolC.tile([128, 8], mybir.dt.uint32, tag="ix8")
                nc.vector.max_with_indices(mx8, ix8, plog)
                nc.vector.tensor_copy(pseudo_i[:, bt:bt + 1], ix8[:, 0:1])
            # export pseudo (as f32) for host-side exact mask counts
            nc.vector.tensor_copy(stats_sb[:, 16:24], pseudo_i)

        # =========== Phase E: classification head (own rows) ============
        wc_sb = persist.tile([128, ET, NCLS], F16)
        nc.sync.dma_start(out=wc_sb, in_=wc.rearrange("(t p) c -> p t c", p=128))
        bcls_bc = persist.tile([128, NCLS], F32)
        nc.gpsimd.dma_start(
            out=bcls_bc,
            in_=bass.AP(tensor=bcls.ap().tensor, offset=0,
                        ap=[[0, 128]] + list(bcls.ap().ap)))
        clab_sb = persist.tile([128, 1], F32)
        nc.sync.dma_start(out=clab_sb, in_=clab.rearrange("(p o) -> p o", o=1))
        with tc.tile_pool(name="psE", bufs=1, space="PSUM") as psE, \
             tc.tile_pool(name="poolE", bufs=1) as poolE:
            cls_ps = psE.tile([128, NCLS], F32, tag="cls")
            for et in range(ET):
                nc.tensor.matmul(cls_ps, lhsT=hq_sb[:, et, :], rhs=wc_sb[:, et, :],
                                 start=(et == 0), stop=(et == ET - 1))
            coarse = poolE.tile([128, NCLS], F32, tag="coarse")
            nc.vector.tensor_add(coarse, cls_ps, bcls_bc)
            mxc = poolE.tile([128, 1], F32, tag="mxc")
            nc.vector.tensor_reduce(mxc, coarse, axis=AX.X, op=OP.max)
            mxn = poolE.tile([128, 1], F32, tag="mxn")
            nc.vector.tensor_scalar_mul(mxn, mxc, -1.0)
            es = poolE.tile([128, NCLS], BF16, tag="es")
            sume = poolE.tile([128, 1], F32, tag="sume")
            nc.scalar.activation(es, coarse, AF.Exp, bias=mxn[:, 0:1], scale=1.0,
                                 accum_out=sume)
            lse = poolE.tile([128, 1], F32, tag="lse")
            nc.scalar.activation(lse, sume, AF.Ln, bias=0.0, scale=1.0)
            ohc = poolE.tile([128, NCLS], F32, tag="ohc")
            nc.vector.tensor_scalar(ohc, iota_i[:, 0:NCLS], clab_sb, None,
                                    OP.is_equal)
            scrc = poolE.tile([128, NCLS], F32, tag="ohscr")
            picked = poolE.tile([128, 1], F32, tag="picked")
            nc.vector.tensor_mul(scrc, coarse, ohc)
            nc.vector.tensor_reduce(picked, scrc, axis=AX.X, op=OP.add)
            t1 = poolE.tile([128, 1], F32, tag="t1")
            nc.vector.tensor_sub(t1, picked, mxc)
            nc.vector.tensor_sub(stats_sb[:, 24:25], t1, lse)

        # =========== Phase D: the B x KS similarity / mae / mask loop =======
        acc_ms = persist.tile([128, NBT * NCH], F32)   # masked mae sums
        acc_mt = persist.tile([128, NBT * NCH], F32)   # total mae sums
        with tc.tile_pool(name="psD", bufs=2, space="PSUM") as psD, \
             tc.tile_pool(name="poolD", bufs=4) as poolD, \
             tc.tile_pool(name="outD", bufs=4) as outD:
            for ch in range(NCH):
                c0 = ch * CH
                qt = poolD.tile([128, CH], F16, tag="qt")
                nc.sync.dma_start(out=qt, in_=qslT[:, c0:c0 + CH])
                ct = poolD.tile([128, CH], F16, tag="ct")
                nc.sync.dma_start(out=ct, in_=cslT[:, c0:c0 + CH])
                labb = poolD.tile([128, CH], F32, tag="labb")
                lab_sl = labs[c0:c0 + CH]
                nc.gpsimd.dma_start(
                    out=labb,
                    in_=bass.AP(tensor=lab_sl.tensor, offset=lab_sl.offset,
                                ap=[[0, 128]] + list(lab_sl.ap)))
                for bt in range(NBT):
                    bsl = slice(bt * 128, (bt + 1) * 128)
                    ci = bt * NCH + ch
                    sq_ps = psD.tile([128, CH], F32, tag="sq")
                    nc.tensor.matmul(sq_ps[:, 0:512], lhsT=q16q[:, bsl],
                                     rhs=qt[:, 0:512])
                    nc.tensor.matmul(sq_ps[:, 512:CH], lhsT=q16q[:, bsl],
                                     rhs=qt[:, 512:CH])
                    lgch = outD.tile([128, CH], F32, tag="lgch")
                    if (ch * NBT + bt) % 2 == 0:
                        nc.vector.tensor_copy(lgch, sq_ps)
                    else:
                        nc.scalar.copy(lgch, sq_ps)
                    nc.sync.dma_start(out=lneg[bsl, c0:c0 + CH], in_=lgch)
                    sc_ps = psD.tile([128, CH], F32, tag="sc")
                    nc.tensor.matmul(sc_ps[:, 0:512], lhsT=q16c[:, bsl],
                                     rhs=ct[:, 0:512])
                    nc.tensor.matmul(sc_ps[:, 512:CH], lhsT=q16c[:, bsl],
                                     rhs=ct[:, 512:CH])
                    mae = poolD.tile([128, CH], BF16, tag="mae")
                    nc.scalar.activation(mae, sc_ps, AF.Sqrt,
                                         bias=maebias[:, 0:1], scale=1.0,
                                         accum_out=acc_mt[:, ci:ci + 1])
                    scr = poolD.tile([128, CH], BF16, tag="scr")
                    nc.vector.scalar_tensor_tensor(
                        out=scr, in0=labb, scalar=pseudo_i[:, bt:bt + 1],
                        in1=mae, op0=OP.is_equal, op1=OP.mult,
                        accum_out=acc_ms[:, ci:ci + 1])

            # =========== Final: fold chunk accumulators, write stats ========
            for bt in range(NBT):
                sl = slice(bt * NCH, (bt + 1) * NCH)
                nc.vector.tensor_reduce(stats_sb[:, bt:bt + 1], acc_ms[:, sl],
                                        axis=AX.X, op=OP.add)
                nc.vector.tensor_reduce(stats_sb[:, 8 + bt:9 + bt], acc_mt[:, sl],
                                        axis=AX.X, op=OP.add)
            nc.scalar.dma_start(out=stats.ap(), in_=stats_sb)

    nc.compile()
    return nc


_NC_CACHE = None


def _get_nc():
    global _NC_CACHE
    if _NC_CACHE is None:
        _NC_CACHE = build_kernel()
    return _NC_CACHE


def kernel(h_q, h_k, W_cls, b_cls, W_p1, b_p1, W_p2, b_p2,
           Wk_p1, bk_p1, Wk_p2, bk_p2, queue_emb, queue_emb_copy,
           info_label, coarse_labs):
    nc = _get_nc()
    import ml_dtypes
    bf16_t = ml_dtypes.bfloat16
    f16 = np.float16
    f32 = np.float32
    ca = np.ascontiguousarray

    h_q = np.asarray(h_q, f32)
    h_k = np.asarray(h_k, f32)
    queue_emb = np.asarray(queue_emb, f32)
    queue_emb_copy = np.asarray(queue_emb_copy, f32)
    info_label = np.asarray(info_label).astype(np.int32)
    coarse_labs = np.asarray(coarse_labs).astype(np.int32)

    w1 = ca(np.asarray(W_p1, f16))
    wk1 = ca(np.asarray(Wk_p1, f16))
    w2 = ca(np.asarray(W_p2, f16))
    wk2 = ca(np.asarray(Wk_p2, f16))
    wc = ca(np.asarray(W_cls, f16))
    bp1 = ca(np.asarray(b_p1, f16))
    bkp1 = ca(np.asarray(bk_p1, f16))
    bp2 = ca(np.asarray(b_p2, f32))
    bkp2 = ca(np.asarray(bk_p2, f32))
    bcv = ca(np.asarray(b_cls, f32))
    queueT = ca(queue_emb.T.astype(f16))        # [D, K]
    copyT = ca(queue_emb_copy.T.astype(f16))    # [D, K]

    in_maps = []
    for c in range(NCORES):
        bs = slice(c * BS, (c + 1) * BS)
        ks = slice(c * KS, (c + 1) * KS)
        in_maps.append({
            "hqT": ca(h_q[bs].T.astype(f16)),
            "hkT": ca(h_k[bs].T.astype(f16)),
            "w1": w1, "wk1": wk1, "w2": w2, "wk2": wk2, "wc": wc,
            "bp1": bp1, "bkp1": bkp1, "bp2": bp2, "bkp2": bkp2, "bcls": bcv,
            "qslT": ca(queueT[:, ks]),
            "cslT": ca(copyT[:, ks]),
            "cnat": ca(queue_emb_copy[ks].astype(bf16_t)),
            "labs": ca(info_label[ks].astype(f32)),
            "clab": ca(coarse_labs[bs].astype(f32)),
        })

    res = run_bass_kernel_spmd(nc, in_maps, list(range(NCORES)))
    results = res.results

    # ---- host-side gather / final scalar math ----
    logits = np.empty((B, 1 + K), f32)
    for c in range(NCORES):
        logits[:, 1 + c * KS: 1 + (c + 1) * KS] = results[c]["lneg"]
    stats = [results[c]["stats"] for c in range(NCORES)]
    for c in range(NCORES):
        logits[c * BS:(c + 1) * BS, 0] = stats[c][:, 25]

    # masked mae sums / total mae sums: partial over each core's K slice
    msum = np.zeros((B,), np.float64)
    mtot = np.zeros((B,), np.float64)
    for c in range(NCORES):
        msum += stats[c][:, 0:8].T.reshape(B).astype(np.float64)
        mtot += stats[c][:, 8:16].T.reshape(B).astype(np.float64)
    # pseudo labels (identical on every core; take core 0)
    pseudo = stats[0][:, 16:24].T.reshape(B).astype(np.int64)
    class_counts = np.bincount(info_label, minlength=NI)
    cnt = class_counts[pseudo].astype(np.float64)

    eps = 1e-6
    min_e = np.mean(msum / (cnt + eps))
    avg_inter = np.mean((mtot - msum) / (K - cnt + eps))
    dino_loss = np.float32(min_e + (2.0 - avg_inter))

    logp = np.concatenate([stats[c][:, 24] for c in range(NCORES)])
    cls_loss = np.float32(-logp.mean())

    labels = np.zeros((B,), np.int32)
    return logits, labels, dino_loss, cls_loss
